# revision 1
# baseline (speedup 1.0000x reference)
"""Trainium2 Bass kernel for nn_Attention_18726057410699 (gnn_message_passing).

Math (per sample b):
  y        = local_feats[b] @ W_apair                       # [192, 256]
  binv     = binary_feats[b] @ W_bin + b_bin                # [128,128,256]
  z[i,j,k] = y[i,k] + y[j,k] + (binv[i,j,k] if i<128 and j<128 else 0)
  s[i,j]   = sigmoid( sum_k relu(z[i,j,k]) * w_att[k] + b_att )
  out[i,h] = sum_j s[i,j] * local_feats[b][j,h]

Sharding: data-parallel over batch B=8 -> 8 cores, one sample each.
Host prep is layout-only: per-sample transpose of binary_feats to put the
contraction channel on SBUF partitions, plus tiny weight reshapes.
"""

import numpy as np

B, N, H, L, C = 8, 192, 256, 128, 112
NIJ = L * L  # 16384
IB = 4  # i-rows per burst

_CACHE = {}


ENGINE_SEM = {
    "EngineType.PE": "PE_",
    "EngineType.DVE": "DVE_",
    "EngineType.Activation": "Activation_",
    "EngineType.Pool": "Pool_",
    "EngineType.SP": "SP_",
}


def _fix_sync_waits(nc):
    """walrus in this toolchain accepts at most ONE sync-wait per compute
    instruction.  Tile emits several.  Two safe rewrites:
      1. drop self waits (instruction waiting on its own engine/queue sem --
         always satisfied by in-order execution of per-proc sems);
      2. push overflow waits onto earlier same-engine instructions (waiting
         earlier on the same in-order engine is strictly more conservative).
    """
    import dataclasses
    from collections import defaultdict

    il = [i for i in nc.all_instructions()]
    streams = defaultdict(list)
    for inst in il:
        si = getattr(inst, "sync_info", None)
        if si is None:
            continue
        upd = {u.ant_name for u in si.on_update}
        eng = str(getattr(inst, "engine", None))
        self_pfx = ENGINE_SEM.get(eng)
        keep = {}
        for w in si.on_wait:
            if w.ant_name in upd:
                continue  # self queue/engine sem
            if self_pfx and w.ant_name.startswith(self_pfx):
                continue  # own engine sem
            k = w.ant_name
            if k not in keep or keep[k].wait_value < w.wait_value:
                keep[k] = w
        new = list(keep.values())
        if len(new) != len(si.on_wait):
            inst.sync_info = dataclasses.replace(si, on_wait=new)
        if type(inst).__name__ in (
            "InstMatmult", "InstTensorCopy", "InstTensorTensor",
            "InstTensorScalarPtr", "InstActivation", "InstMemset",
            "InstTensorReduce", "InstTensorTensorReduce",
        ):
            streams[eng].append(inst)

    for eng, insts in streams.items():
        overflow = []
        for inst in reversed(insts):
            si = inst.sync_info
            waits = list(si.on_wait) + overflow
            ded = {}
            for w in waits:
                if w.ant_name not in ded or ded[w.ant_name].wait_value < w.wait_value:
                    ded[w.ant_name] = w
            waits = list(ded.values())
            if len(waits) <= 1:
                inst.sync_info = dataclasses.replace(si, on_wait=waits)
                overflow = []
            else:
                inst.sync_info = dataclasses.replace(si, on_wait=[waits[-1]])
                overflow = waits[:-1]
        if overflow:
            raise RuntimeError(f"{eng}: could not place {len(overflow)} waits")


def _build():
    import concourse.bass as bass
    import concourse.tile as tile
    from concourse import bacc, mybir

    f32 = mybir.dt.float32
    bf16 = mybir.dt.bfloat16
    ALU = mybir.AluOpType
    ACTF = mybir.ActivationFunctionType

    nc = bacc.Bacc()

    p_binT = nc.declare_dram_parameter("binT", [C, NIJ], f32, isOutput=False)
    p_xw = nc.declare_dram_parameter("xw", [128, 2, N + H], f32, isOutput=False)
    p_x = nc.declare_dram_parameter("x", [N, H], f32, isOutput=False)
    p_wbin = nc.declare_dram_parameter("wbin", [C, H], f32, isOutput=False)
    p_bbin = nc.declare_dram_parameter("bbin", [128, 2], f32, isOutput=False)
    p_watt = nc.declare_dram_parameter("watt", [128, 2], f32, isOutput=False)
    p_batt = nc.declare_dram_parameter("battc", [128, 1], f32, isOutput=False)
    p_eye = nc.declare_dram_parameter("eye", [128, 128], f32, isOutput=False)
    p_e4 = nc.declare_dram_parameter("e4", [IB, IB * L], f32, isOutput=False)
    p_out = nc.declare_dram_parameter("out", [N, H], f32, isOutput=True)

    with tile.TileContext(nc) as tc:
        with (
            tc.tile_pool(name="const", bufs=1) as cpool,
            tc.tile_pool(name="binchunk", bufs=3) as bcpool,
            tc.tile_pool(name="work", bufs=3) as wpool,
            tc.tile_pool(name="uwork", bufs=4) as upool,
            tc.tile_pool(name="srow", bufs=4) as srpool,
            tc.tile_pool(name="fin", bufs=1) as fpool,
            tc.tile_pool(name="pbin", bufs=6, space=bass.MemorySpace.PSUM) as pbpool,
            tc.tile_pool(name="pscore", bufs=2, space=bass.MemorySpace.PSUM) as pspool,
            tc.tile_pool(name="dram", bufs=1, space=bass.MemorySpace.DRAM) as dpool,
        ):
            # ---------------- constants / prep ----------------
            binT_bf = cpool.tile([C, NIJ], bf16, tag="binTbf")
            CH = 2048

            bb_sb = cpool.tile([128, 2], f32, tag="bb")
            nc.sync.dma_start(bb_sb[:, :], p_bbin[:, :])

            wa_sb = cpool.tile([128, 2], f32, tag="wa")
            nc.sync.dma_start(wa_sb[:, :], p_watt[:, :])

            xw_sb = cpool.tile([128, 2, N + H], f32, tag="xw")  # [p, h-tile, j | k]
            nc.sync.dma_start(xw_sb[:, 0, :], p_xw[:, 0, :])
            nc.sync.dma_start(xw_sb[:, 1, :], p_xw[:, 1, :])

            wbin_f = cpool.tile([C, H], f32, tag="wbinf")
            nc.sync.dma_start(wbin_f[:, :], p_wbin[:, :])
            wbin_bf = cpool.tile([C, H], bf16, tag="wbinbf")
            nc.scalar.copy(wbin_bf[:, :], wbin_f[:, :])
            # W4: per (kt, m) a [128,4] stationary with watt[kt] in col m, zeros else
            w4 = cpool.tile([128, 2, 2 * IB, 2 * IB], bf16, tag="w4")
            nc.vector.memset(w4[:, :, :, :], 0.0)
            for kt in range(2):
                for m in range(2 * IB):
                    nc.vector.tensor_copy(w4[:, kt, m, m : m + 1], wa_sb[:, kt : kt + 1])

            bc_sb = cpool.tile([128, 1], f32, tag="battc")
            nc.sync.dma_start(bc_sb[:, :], p_batt[:, :])
            sigwarm = cpool.tile([1, 1], bf16, tag="sigwarm")
            nc.scalar.activation(sigwarm[:, :], bc_sb[0:1, 0:1], ACTF.Sigmoid, bias=0.0, scale=1.0)

            x_f0 = cpool.tile([128, H], f32, tag="xf0")
            x_f1 = cpool.tile([64, H], f32, tag="xf1")
            nc.sync.dma_start(x_f0[:, :], p_x[0:128, :])
            nc.sync.dma_start(x_f1[:, :], p_x[128:192, :])
            x_bf0 = cpool.tile([128, H], bf16, tag="xbf0")
            x_bf1 = cpool.tile([64, H], bf16, tag="xbf1")
            nc.scalar.copy(x_bf0[:, :], x_f0[:, :])
            nc.scalar.copy(x_bf1[:, :], x_f1[:, :])

            # Y^T[k,j] = sum_h Wap[h,k] * XT[h,j]   (per k-tile)
            yt_f = cpool.tile([128, 2 * N], f32, tag="ytf")  # fp32, per-i bias source
            yt_b = cpool.tile([128, 2 * N], bf16, tag="ytb")  # bf16, region-B stream source
            base4 = cpool.tile([128, 2, IB, L], bf16, tag="base4")  # (Y^T + b_bin)[:, :128] x4
            for kt in range(2):
                psy = pspool.tile([128, H], f32, tag="score")
                for ht in range(2):
                    nc.tensor.matmul(
                        psy[:, 0:N],
                        xw_sb[:, ht, N + kt * 128 : N + (kt + 1) * 128],
                        xw_sb[:, ht, 0:N],
                        start=(ht == 0),
                        stop=(ht == 1),
                    )
                nc.vector.tensor_copy(yt_f[:, kt * N : (kt + 1) * N], psy[:, 0:N])
                nc.scalar.copy(yt_b[:, kt * N : (kt + 1) * N], psy[:, 0:N])
                nc.vector.tensor_scalar(
                    base4[:, kt, 0, :], psy[:, 0:L], bb_sb[:, kt : kt + 1], None, ALU.add
                )
                for g in range(1, IB):
                    nc.vector.tensor_copy(base4[:, kt, g, :], base4[:, kt, 0, :])

            eye_f = cpool.tile([128, 128], f32, tag="eyef")
            nc.sync.dma_start(eye_f[:, :], p_eye[:, :])
            eye_b = cpool.tile([128, 128], bf16, tag="eyeb")
            nc.scalar.copy(eye_b[:, :], eye_f[:, :])
            e4_f = cpool.tile([IB, IB * L], f32, tag="e4f")
            nc.sync.dma_start(e4_f[:, :], p_e4[:, :])
            e4_b = cpool.tile([IB, IB, L], bf16, tag="e4b")
            nc.scalar.copy(e4_b[:, :, :], e4_f[:, :])

            # Y in natural layout (rows i<128 on partitions) for the bias-fold matmul
            yn0 = cpool.tile([128, 2 * 128], bf16, tag="yn0")
            for kt in range(2):
                pst = pspool.tile([128, 128], bf16, tag="score")
                nc.tensor.transpose(pst[:, :], yt_b[:, kt * N : kt * N + 128], eye_b[:, :])
                nc.vector.tensor_copy(yn0[:, kt * 128 : (kt + 1) * 128], pst[:, :])
            # regroup: ynG[r, ib, :] = yn0[ib*4+r, :] so burst stationaries sit at partitions 0..3
            ynG = cpool.tile([IB, 32, 2 * 128], bf16, tag="ynG")
            for ib in range(32):
                nc.gpsimd.dma_start(ynG[:, ib, :], yn0[ib * IB : (ib + 1) * IB, :])

            # binT load + cast to bf16
            for ch in range(NIJ // CH):
                bchunk = bcpool.tile([C, CH], f32, tag="bchunk")
                nc.sync.dma_start(bchunk[:, :], p_binT[:, ch * CH : (ch + 1) * CH])
                nc.scalar.copy(binT_bf[:, ch * CH : (ch + 1) * CH], bchunk[:, :])

            scratch = dpool.tile([N, N], bf16, tag="scratch")

            # ---------------- main loops ----------------
            lo_state = {}
            hi_state = {}

            def score_rows(score_state, ib, i0, u, w):
                """Reduce IB i-rows; scores accumulate in psum stripes; flush every 2 bursts."""
                if ib % 2 == 0:
                    ps_t = pspool.tile([128, H], f32, tag="score")
                    score_state["ps"] = ps_t
                    score_state["first"] = True
                ps = score_state["ps"]
                sb = (ib % 2) * IB
                for kt in range(2):
                    for m in range(IB):
                        nc.tensor.matmul(
                            ps[0 : 2 * IB, 0:w],
                            w4[:, kt, sb + m, :],
                            u[:, kt, m, 0:w],
                            start=score_state["first"],
                            stop=(ib % 2 == 1 and kt == 1 and m == IB - 1),
                            skip_group_check=True,
                        )
                        score_state["first"] = False
                if ib % 2 == 1:
                    srow = srpool.tile([2 * IB, N], bf16, tag="srow")
                    nc.scalar.activation(
                        srow[:, 0:w], ps[0 : 2 * IB, 0:w], ACTF.Sigmoid,
                        bias=bc_sb[0 : 2 * IB, 0:1], scale=1.0,
                    )
                    nc.gpsimd.dma_start(scratch[i0 - IB : i0 + IB, 0:w], srow[:, 0:w])


            st0 = fpool.tile([128, N], bf16, tag="st0")
            st1 = fpool.tile([64, N], bf16, tag="st1")

            def hi_burst(ib):
                i0 = L + ib * IB
                u = upool.tile([128, 2, IB, N], bf16, tag="uhi")
                for kt in range(2):
                    for m in range(IB):
                        yi = yt_f[:, kt * N + i0 + m : kt * N + i0 + m + 1]
                        nc.vector.tensor_scalar(
                            u[:, kt, m, :],
                            yt_b[:, kt * N : (kt + 1) * N],
                            yi,
                            0.0,
                            ALU.add,
                            ALU.max,
                        )
                score_rows(hi_state, ib, i0, u, N)


            def lo_burst(ib):
                i0 = ib * IB
                pb = []
                for kt in range(2):
                    pbt = pbpool.tile([128, IB, L], f32, tag="pb")
                    for m in range(IB):
                        nc.tensor.matmul(
                            pbt[:, m, :],
                            wbin_bf[:, kt * 128 : (kt + 1) * 128],
                            binT_bf[:, (i0 + m) * L : (i0 + m + 1) * L],
                            start=(m == 0),
                            stop=False,
                            skip_group_check=True,
                        )
                    nc.tensor.matmul(
                        pbt[:, :, :],
                        ynG[:, ib, kt * 128 : (kt + 1) * 128],
                        e4_b[:, :, :],
                        start=False,
                        stop=True,
                        skip_group_check=True,
                    )
                    pb.append(pbt)
                z = wpool.tile([128, 2, IB, L], bf16, tag="z")
                zc = wpool.tile([128, IB, L], bf16, tag="zc")
                nc.scalar.copy(zc[:, :, :], pb[1][:, :, :])
                nc.vector.tensor_add(z[:, 1, :, :], zc[:, :, :], base4[:, 1, :, :])
                if ib >= 29:
                    zc2 = wpool.tile([128, IB, L], bf16, tag="zc2")
                    nc.scalar.copy(zc2[:, :, :], pb[0][:, :, :])
                    nc.vector.tensor_add(z[:, 0, :, :], zc2[:, :, :], base4[:, 0, :, :])
                else:
                    nc.vector.tensor_add(z[:, 0, :, :], pb[0][:, :, :], base4[:, 0, :, :])
                u = upool.tile([128, 2, IB, L], bf16, tag="u")
                nc.vector.tensor_scalar(
                    u[:, :, :, :], z[:, :, :, :], 0.0, None, ALU.max
                )
                score_rows(lo_state, ib, i0, u, L)



            for _h in range(7):
                hi_burst(_h)
            hi_next = 7
            for ck in range(8):
                for k in range(4):
                    lo_burst(ck * 4 + k)
                    if k % 2 == 1 and hi_next < 16:
                        hi_burst(hi_next)
                        hi_next += 1

            # hi rows complete: fill every score block that depends on them
            nc.sync.dma_start_transpose(st0[:, L:N], scratch[L:N, 0:L])
            nc.sync.dma_start(st1[:, 0:L], scratch[L:N, 0:L])
            nc.sync.dma_start_transpose(st1[:, L:N], scratch[L:N, L:N])

            # ---------------- finale: last transpose, sigmoid, S @ X ----------------
            nc.sync.dma_start_transpose(st0[:, 0:L], scratch[0:L, 0:L])

            for it, (lo, sz) in enumerate(((0, 128), (128, 64))):
                po = pspool.tile([128, H], f32, tag="score")
                nc.tensor.matmul(po[0:sz, :], st0[:, lo : lo + sz], x_bf0[:, :], start=True, stop=False)
                nc.tensor.matmul(po[0:sz, :], st1[:, lo : lo + sz], x_bf1[:, :], start=False, stop=True)
                ob = fpool.tile([sz, H], f32, tag=f"ob{it}")
                nc.vector.tensor_copy(ob[:, :], po[0:sz, :])
                nc.sync.dma_start(p_out[lo : lo + sz, :], ob[:, :])

    nc.compile()
    return nc


def _e4_const():
    e = np.zeros((IB, IB, L), np.float32)
    for m in range(IB):
        e[m, m, :] = 1.0
    return np.ascontiguousarray(e.reshape(IB, IB * L))


def _prep_inputs(local_feats, binary_feats, W_apair, W_bin, b_bin, w_att, b_att):
    lf = np.asarray(local_feats, np.float32)
    bf = np.asarray(binary_feats, np.float32)
    wap = np.ascontiguousarray(np.asarray(W_apair, np.float32))
    wbin = np.ascontiguousarray(np.asarray(W_bin, np.float32))
    bb = np.ascontiguousarray(np.asarray(b_bin, np.float32).reshape(H, 1))
    wa = np.ascontiguousarray(np.asarray(w_att, np.float32).reshape(H, 1))
    battc = np.full((128, 1), np.float32(np.asarray(b_att).reshape(-1)[0]), np.float32)
    in_maps = []
    for b in range(B):
        in_maps.append(
            {
                "binT": np.ascontiguousarray(bf[b].reshape(NIJ, C).T),
                "xw": np.ascontiguousarray(np.concatenate([
                    lf[b].T.reshape(2, 128, N).transpose(1, 0, 2),
                    wap.reshape(2, 128, H).transpose(1, 0, 2)], axis=2)),
                "x": np.ascontiguousarray(lf[b]),
                "wbin": wbin,
                "bbin": np.ascontiguousarray(bb.reshape(2, 128).T),
                "watt": np.ascontiguousarray(wa.reshape(2, 128).T),
                "battc": battc,
                "eye": np.eye(128, dtype=np.float32),
                "e4": _e4_const(),
            }
        )
    return in_maps


def run_full(inputs, trace=False):
    from concourse.bass_utils import run_bass_kernel_spmd

    if "nc" not in _CACHE:
        _CACHE["nc"] = _build()
    nc = _CACHE["nc"]
    in_maps = _prep_inputs(
        inputs["local_feats"],
        inputs["binary_feats"],
        inputs["W_apair"],
        inputs["W_bin"],
        inputs["b_bin"],
        inputs["w_att"],
        inputs["b_att"],
    )
    res = run_bass_kernel_spmd(nc, in_maps, list(range(B)), trace=trace)
    out = np.stack([np.asarray(res.results[c]["out"], np.float32) for c in range(B)])
    return out, res


def kernel(**inputs):
    out, _ = run_full(inputs, trace=False)
    return out



# revision 27
# speedup vs baseline: 1.3399x; 1.3399x over previous
"""Trainium2 Bass kernel for nn_Attention_18726057410699 (gnn_message_passing).

Math (per sample b):
  Y        = X @ W_apair                                  # [192, 256]
  z[i,j,k] = Y[i,k] + Y[j,k] + (binv[i,j,k] + bb[k] if i<128 and j<128)
  s[i,j]   = sigmoid( sum_k relu(z[i,j,k]) * watt[k] + batt )
  out[i,h] = sum_j s[i,j] * X[j,h]

Key structure (all per core; data-parallel over batch B=8 -> 8 cores):
  * binT_x [116, 16384] bf16 (host-staged): binT rows + 4 one-hot rows so a
    single matmul per (burst, kt) yields binv + Y[i] + bb in psum (the Y/bb
    values sit in stationary rows 112..115, fed by an on-device transpose
    via a DRAM scratch roundtrip).
  * max-trick: relu(p + Y[j]) = max(p, -Y[j]) + Y[j]; the sum_k w*Y[j] term
    is added as a rank-1 matmul into the score psum. One fused elementwise
    op per psum tile instead of add+relu.
  * scores via fp8 DoubleRow matmuls (0.5 cyc/row, both k-tiles at once),
    watt pre-scaled x8 on host; sigmoid applies scale=1/8.
  * hi rows (i>=128, no binv): u = relu(Y[j]+Y[i]) via fused tensor_scalar /
    activation, spread across DVE/Act/Pool.
  * S^T assembled with on-chip PE transposes (no DRAM scratch for scores).
"""

import numpy as np
import ml_dtypes

B, N, H, L, C = 8, 192, 256, 128, 112
NBLO, NBHI = 32, 16  # bursts of 4 i-rows

_CACHE = {}


ENGINE_SEM = {
    "EngineType.PE": "PE_",
    "EngineType.DVE": "DVE_",
    "EngineType.Activation": "Activation_",
    "EngineType.Pool": "Pool_",
    "EngineType.SP": "SP_",
}


def _fix_sync_waits(nc):
    """walrus accepts at most ONE sync-wait per compute instruction; Tile
    emits several.  Drop self waits, push overflow onto earlier same-engine
    instructions (strictly more conservative)."""
    import dataclasses
    from collections import defaultdict

    il = [i for i in nc.all_instructions()]
    streams = defaultdict(list)
    for inst in il:
        si = getattr(inst, "sync_info", None)
        if si is None:
            continue
        upd = {u.ant_name for u in si.on_update}
        eng = str(getattr(inst, "engine", None))
        self_pfx = ENGINE_SEM.get(eng)
        keep = {}
        for w in si.on_wait:
            if w.ant_name in upd:
                continue
            if self_pfx and w.ant_name.startswith(self_pfx):
                continue
            k = w.ant_name
            if k not in keep or keep[k].wait_value < w.wait_value:
                keep[k] = w
        new = list(keep.values())
        if len(new) != len(si.on_wait):
            inst.sync_info = dataclasses.replace(si, on_wait=new)
        if type(inst).__name__ in (
            "InstMatmult", "InstTensorCopy", "InstTensorTensor",
            "InstTensorScalarPtr", "InstActivation", "InstMemset",
            "InstTensorReduce", "InstTensorTensorReduce",
        ):
            streams[eng].append(inst)

    for eng, insts in streams.items():
        overflow = []
        for inst in reversed(insts):
            si = inst.sync_info
            waits = list(si.on_wait) + overflow
            ded = {}
            for w in waits:
                if w.ant_name not in ded or ded[w.ant_name].wait_value < w.wait_value:
                    ded[w.ant_name] = w
            waits = list(ded.values())
            if len(waits) <= 1:
                inst.sync_info = dataclasses.replace(si, on_wait=waits)
                overflow = []
            else:
                inst.sync_info = dataclasses.replace(si, on_wait=[waits[-1]])
                overflow = waits[:-1]
        if overflow:
            raise RuntimeError(f"{eng}: could not place {len(overflow)} waits")


def _build():
    import concourse.bass as bass
    import concourse.tile as tile
    from concourse import bacc, mybir

    f32 = mybir.dt.float32
    bf16 = mybir.dt.bfloat16
    fp8 = mybir.dt.float8e4
    ALU = mybir.AluOpType
    ACTF = mybir.ActivationFunctionType
    DR = mybir.MatmulPerfMode.DoubleRow

    nc = bacc.Bacc()

    p_binT = nc.declare_dram_parameter("binT", [116, L * L], bf16, isOutput=False)
    p_xw = nc.declare_dram_parameter("xw", [128, 2, N + H], f32, isOutput=False)
    p_xbf0 = nc.declare_dram_parameter("xbf0", [128, H], bf16, isOutput=False)
    p_xbf1 = nc.declare_dram_parameter("xbf1", [64, H], bf16, isOutput=False)
    p_w4dm = nc.declare_dram_parameter("w4dm", [128, 2, 32, 32], fp8, isOutput=False)
    p_wbin = nc.declare_dram_parameter("wbin", [112, 2, 128], bf16, isOutput=False)
    p_w8b = nc.declare_dram_parameter("w8b", [128, 2, 1], bf16, isOutput=False)
    p_bb = nc.declare_dram_parameter("bbcol", [128, 2, 1], f32, isOutput=False)
    p_batt = nc.declare_dram_parameter("battc", [32, 1], f32, isOutput=False)
    p_eye = nc.declare_dram_parameter("eyeb", [128, 128], bf16, isOutput=False)
    p_out = nc.declare_dram_parameter("out", [N, H], f32, isOutput=True)

    # Pool cannot touch PSUM (walrus verifier), so lo units are DVE max-trick
    # or Act-path (PE folds y_j into psum, Act does relu+fp8 from psum).
    # Slot = (group g 0..7, kt): Act-path slots chosen to balance engines.
    ACT_SLOTS = {(g, 1) for g in range(1, 8)}  # 7 slots = 28 kt-half units
    # hi op engine per (kt*4 + m) within a burst: Pool (all-SBUF legal)
    HI_ENG = ["P", "P", "P", "P", "P", "P", "P", "P"]

    with tile.TileContext(nc) as tc:
        with (
            tc.tile_pool(name="const", bufs=1) as cpool,
            tc.tile_pool(name="ulo", bufs=3) as ulopool,
            tc.tile_pool(name="uhi", bufs=3) as uhipool,
            tc.tile_pool(name="pbin0", bufs=2, space=bass.MemorySpace.PSUM) as pb0pool,
            tc.tile_pool(name="pbin1", bufs=2, space=bass.MemorySpace.PSUM) as pb1pool,
            tc.tile_pool(name="pscore", bufs=2, space=bass.MemorySpace.PSUM) as pspool,
            tc.tile_pool(name="pmisc", bufs=1, space=bass.MemorySpace.PSUM) as pmpool,
            tc.tile_pool(name="dram", bufs=1, space=bass.MemorySpace.DRAM) as dpool,
        ):
            # ---------------- param loads ----------------
            binT_x = cpool.tile([116, L * L], bf16, tag="binTx")
            CH = 2048
            for ch in range(8):
                nc.sync.dma_start(binT_x[:, ch * CH:(ch + 1) * CH],
                                  p_binT[:, ch * CH:(ch + 1) * CH])

            xw_sb = cpool.tile([128, 2, N + H], f32, tag="xw")
            nc.scalar.dma_start(xw_sb[:, :, :], p_xw[:, :, :])
            eye_b = cpool.tile([128, 128], bf16, tag="eyeb")
            nc.scalar.dma_start(eye_b[:, :], p_eye[:, :])
            w4dm = cpool.tile([128, 2, 32, 32], fp8, tag="w4dm")
            nc.scalar.dma_start(w4dm[:, :, :, :], p_w4dm[:, :, :, :])
            bc_sb = cpool.tile([32, 1], f32, tag="battc")
            nc.scalar.dma_start(bc_sb[:, :], p_batt[:, :])

            wbin_bf = cpool.tile([112, 2, 128], bf16, tag="wbin")
            nc.gpsimd.dma_start(wbin_bf[:, :, :], p_wbin[:, :, :])
            x_bf0 = cpool.tile([128, H], bf16, tag="xbf0")
            nc.gpsimd.dma_start(x_bf0[:, :], p_xbf0[:, :])
            x_bf1 = cpool.tile([64, H], bf16, tag="xbf1")
            nc.gpsimd.dma_start(x_bf1[:, :], p_xbf1[:, :])
            w8b_sb = cpool.tile([128, 2, 1], bf16, tag="w8b")
            nc.gpsimd.dma_start(w8b_sb[:, :, :], p_w8b[:, :, :])
            bb_sb = cpool.tile([128, 2, 1], f32, tag="bbcol")
            nc.gpsimd.dma_start(bb_sb[:, :, :], p_bb[:, :, :])

            # ---------------- Y and derived tiles ----------------
            yt_f = cpool.tile([128, 2, N], f32, tag="ytf")
            yt_b = cpool.tile([128, 2, N], bf16, tag="ytb")
            ytbb = cpool.tile([128, 2, 128], bf16, tag="ytbb")
            ytN = cpool.tile([128, 2, 128], bf16, tag="ytN")
            for kt in range(2):
                psy = pmpool.tile([128, H], f32, tag="po")
                for ht in range(2):
                    nc.tensor.matmul(
                        psy[:, 0:N],
                        xw_sb[:, ht, N + kt * 128:N + (kt + 1) * 128],
                        xw_sb[:, ht, 0:N],
                        start=(ht == 0), stop=(ht == 1),
                    )
                nc.vector.tensor_copy(yt_f[:, kt, :], psy[:, 0:N])
                nc.scalar.copy(yt_b[:, kt, :], psy[:, 0:N])
            for kt in range(2):
                nc.vector.tensor_scalar(
                    ytbb[:, kt, :], yt_b[:, kt, 0:128], bb_sb[:, kt, 0:1], None, ALU.add)
            nc.vector.tensor_scalar(
                ytN[:, :, :], yt_b[:, :, 0:128], -1.0, None, ALU.mult)

            # sigmoid table warm (Relu/Copy share the sigmoid table)
            sigwarm = cpool.tile([1, 1], bf16, tag="sigwarm")
            nc.scalar.activation(sigwarm[:, :], bc_sb[0:1, 0:1], ACTF.Sigmoid, bias=0.0, scale=1.0)

            # stat_all: [128, 32, 2, 128]; rows 0..111 wbin replicated, rows
            # 112..115 Ypb rows for each burst (via DRAM scratch roundtrip).
            stat_all = cpool.tile([128, NBLO, 2, 128], bf16, tag="statall")
            nc.vector.tensor_copy(stat_all[0:112, 0, :, :], wbin_bf[:, :, :])
            g = 1
            while g < NBLO:
                n = min(g, NBLO - g)
                nc.vector.tensor_copy(stat_all[0:112, g:g + n, :, :],
                                      stat_all[0:112, 0:n, :, :])
                g += n

            scr4 = dpool.tile([4, NBLO, 2, 128], bf16, tag="scr4")
            ypb0 = cpool.tile([128, 2, 128], bf16, tag="ypb0")
            yn0 = cpool.tile([128, 2, 128], bf16, tag="yn0")
            for kt in range(2):
                pT = pmpool.tile([128, 128], bf16, tag="pT")
                nc.tensor.transpose(pT[:, :], ytbb[:, kt, :], eye_b[:, :])
                nc.vector.tensor_copy(ypb0[:, kt, :], pT[:, :])
                # scr4[m, ib, kt, :] = Ypb[4*ib + m, kt-half]; ypb0 row j=4ib+m
                dst = scr4[:, :, kt, :].transpose([1, 0, 2])  # iterate (ib, m, k)
                nc.gpsimd.dma_start(dst, ypb0[:, kt, :])
                # Y natural (no bb) for the Act-path y_j fold
                pT2s = pmpool.tile([128, 128], bf16, tag="pT")
                nc.tensor.transpose(pT2s[:, :], yt_b[:, kt, 0:128], eye_b[:, :])
                nc.vector.tensor_copy(yn0[:, kt, :], pT2s[:, :])
            nc.gpsimd.dma_start(stat_all[112:116, :, :, :], scr4[:, :, :, :])

            # eyer4[j', m, j] = eye[j', j] for all m (Act-path fold moving)
            eyer4 = cpool.tile([128, 4, 128], bf16, tag="eyer4")
            nc.vector.tensor_copy(
                eyer4[:, :, :], eye_b[:, :].unsqueeze(1).broadcast_to([128, 4, 128]))

            # wb8h[kt] = sum_{k in kt half} w8 * Y^T[k, j] (per-half correction)
            # ones32 applies to all 32 group rows; ind0/ind1 to one 16-row half
            ones32 = cpool.tile([1, 32], bf16, tag="ones32")
            nc.vector.memset(ones32[:, :], 1.0)
            ind0 = cpool.tile([1, 32], bf16, tag="ind0")
            nc.vector.memset(ind0[:, :], 0.0)
            nc.vector.memset(ind0[:, 0:16], 1.0)
            ind1 = cpool.tile([1, 32], bf16, tag="ind1")
            nc.vector.memset(ind1[:, :], 0.0)
            nc.vector.memset(ind1[:, 16:32], 1.0)
            wb8 = cpool.tile([1, 2, 128], bf16, tag="wb8")
            pw_t = pmpool.tile([128, H], f32, tag="po")
            for kt in range(2):
                pw = pw_t[0:1, kt * 128:kt * 128 + 128]
                nc.tensor.matmul(pw, w8b_sb[:, kt, :], yt_b[:, kt, 0:128],
                                 start=True, stop=True)
                nc.vector.tensor_copy(wb8[:, kt, :], pw)

            lo_s = cpool.tile([128, 128], bf16, tag="los")
            hi_s = cpool.tile([64, N], bf16, tag="his")

            # ---------------- burst bodies ----------------
            lo_state = {}
            hi_state = {}

            def lo_burst(ib):
                g = ib // 4
                r0 = (ib % 4) * 4
                u = ulopool.tile([128, 2, 4, 128], fp8, tag="ulo")
                for kt in range(2):
                    act_path = (g, kt) in ACT_SLOTS
                    pool = pb0pool if kt == 0 else pb1pool
                    pb = pool.tile([128, 4, 128], f32, tag=f"pb{kt}")
                    nc.tensor.matmul(
                        pb[:, :, :],
                        stat_all[0:116, ib, kt, :],
                        binT_x[0:116, ib * 512:(ib + 1) * 512],
                        start=True, stop=not act_path,
                    )
                    if act_path:
                        # fold y_j into psum, then relu+fp8 on Act
                        nc.tensor.matmul(pb[:, :, :], yn0[:, kt, :], eyer4[:, :, :],
                                         start=False, stop=True)
                        nc.scalar.activation(u[:, kt, :, :], pb[:, :, :],
                                             ACTF.Relu, bias=0.0, scale=1.0)
                    else:
                        ytn_bc = ytN[:, kt, :].unsqueeze(1).broadcast_to([128, 4, 128])
                        nc.vector.tensor_tensor(u[:, kt, :, :], pb[:, :, :], ytn_bc, ALU.max)
                r0g = (ib % 8) * 4
                if ib % 8 == 0:
                    sps_t = pspool.tile([32, N], f32, tag="score")
                    lo_state["ps"] = sps_t
                sps = lo_state["ps"]
                for m in range(4):
                    nc.tensor.matmul(
                        sps[0:32, 0:128], w4dm[:, :, r0g + m, :], u[:, :, m, :],
                        start=(ib % 8 == 0 and m == 0), stop=False,
                        perf_mode=DR, skip_group_check=True,
                    )
                if ib % 8 == 7:
                    # wb correction per (kt, group-half) for max-trick halves
                    g2 = ib // 8
                    mms = []
                    for kt in range(2):
                        halves = [h for h in (2 * g2, 2 * g2 + 1)
                                  if (h, kt) not in ACT_SLOTS]
                        if len(halves) == 2:
                            mms.append((ones32, kt))
                        elif halves == [2 * g2]:
                            mms.append((ind0, kt))
                        elif halves == [2 * g2 + 1]:
                            mms.append((ind1, kt))
                    for ci, (ind, kt) in enumerate(mms):
                        nc.tensor.matmul(sps[0:32, 0:128], ind[:, :], wb8[:, kt, :],
                                         start=False, stop=(ci == len(mms) - 1),
                                         skip_group_check=True)
                    nc.scalar.activation(
                        lo_s[g2 * 32:(g2 + 1) * 32, :], sps[0:32, 0:128],
                        ACTF.Sigmoid, bias=bc_sb[0:32, 0:1], scale=0.125)

            def hi_burst(hb):
                i0 = 128 + 4 * hb
                r0g = (hb % 8) * 4
                u = uhipool.tile([128, 2, 4, N], fp8, tag="uhi")
                for kt in range(2):
                    for m in range(4):
                        i = i0 + m
                        eng = HI_ENG[kt * 4 + m]
                        if eng == "A":
                            nc.scalar.activation(
                                u[:, kt, m, :], yt_b[:, kt, :], ACTF.Relu,
                                bias=yt_f[:, kt, i:i + 1], scale=1.0)
                        elif eng == "D":
                            nc.vector.tensor_scalar(
                                u[:, kt, m, :], yt_b[:, kt, :],
                                yt_f[:, kt, i:i + 1], 0.0, ALU.add, ALU.max)
                        else:
                            nc.gpsimd.tensor_scalar(
                                u[:, kt, m, :], yt_b[:, kt, :],
                                yt_f[:, kt, i:i + 1], 0.0, ALU.add, ALU.max)
                if hb % 8 == 0:
                    sph_t = pspool.tile([32, N], f32, tag="score")
                    hi_state["ps"] = sph_t
                sph = hi_state["ps"]
                for m in range(4):
                    nc.tensor.matmul(
                        sph[0:32, 0:N], w4dm[:, :, r0g + m, :], u[:, :, m, :],
                        start=(hb % 8 == 0 and m == 0), stop=(hb % 8 == 7 and m == 3),
                        perf_mode=DR, skip_group_check=True,
                    )
                if hb % 8 == 7:
                    gh = hb // 8
                    nc.scalar.activation(
                        hi_s[gh * 32:(gh + 1) * 32, :], sph[0:32, 0:N],
                        ACTF.Sigmoid, bias=bc_sb[0:32, 0:1], scale=0.125)

            # hi bursts need only yt; lo bursts wait on stat_all + binT chunks.
            hi_burst(0)
            hi_burst(1)
            hi_next = 2
            for k in range(16):
                lo_burst(2 * k)
                lo_burst(2 * k + 1)
                if hi_next < NBHI:
                    hi_burst(hi_next)
                    hi_next += 1

            # ---------------- finale: S^T assembly + S @ X ----------------
            st0 = cpool.tile([128, N], bf16, tag="st0")
            st1 = cpool.tile([64, N], bf16, tag="st1")
            pT1 = pmpool.tile([128, 128], bf16, tag="pT")
            nc.tensor.transpose(pT1[:, :], lo_s[:, :], eye_b[:, :])
            nc.vector.tensor_copy(st0[:, 0:128], pT1[:, :])
            pT2_t = pmpool.tile([128, 128], bf16, tag="pT")
            pT2 = pT2_t[:, 0:64]
            nc.tensor.transpose(pT2, hi_s[:, 0:128], eye_b[0:64, 0:64])
            nc.vector.tensor_copy(st0[:, 128:N], pT2)
            pT3_t = pmpool.tile([128, 128], bf16, tag="pT")
            pT3 = pT3_t[0:64, 0:64]
            nc.tensor.transpose(pT3, hi_s[:, 128:N], eye_b[0:64, 0:64])
            nc.vector.tensor_copy(st1[:, 128:N], pT3)
            nc.vector.tensor_copy(st1[:, 0:128], hi_s[:, 0:128])

            for it, (lo, sz) in enumerate(((0, 128), (128, 64))):
                po = pmpool.tile([128, H], f32, tag="po")
                nc.tensor.matmul(po[0:sz, :], st0[:, lo:lo + sz], x_bf0[:, :],
                                 start=True, stop=False)
                nc.tensor.matmul(po[0:sz, :], st1[:, lo:lo + sz], x_bf1[:, :],
                                 start=False, stop=True)
                ob = cpool.tile([sz, H], f32, tag=f"ob{it}")
                nc.vector.tensor_copy(ob[:, :], po[0:sz, :])
                nc.sync.dma_start(p_out[lo:lo + sz, :], ob[:, :])

    nc.compile()
    return nc


def _prep_inputs(local_feats, binary_feats, W_apair, W_bin, b_bin, w_att, b_att):
    bf16 = ml_dtypes.bfloat16
    fp8 = ml_dtypes.float8_e4m3
    lf = np.asarray(local_feats, np.float32)
    bfe = np.asarray(binary_feats, np.float32)
    wap = np.ascontiguousarray(np.asarray(W_apair, np.float32))
    wbin = np.asarray(W_bin, np.float32)
    bb = np.asarray(b_bin, np.float32).reshape(-1)
    wa = np.asarray(w_att, np.float32).reshape(-1)
    batt = np.float32(np.asarray(b_att).reshape(-1)[0])

    w8 = (8.0 * wa).astype(fp8)                      # [256] fp8
    w8f = w8.astype(np.float32)
    # w4dm[k, kt, c_sel, c] = w8[kt*128+k] iff c == c_sel
    w4dm = np.zeros((128, 2, 32, 32), fp8)
    for kt in range(2):
        for c in range(32):
            w4dm[:, kt, c, c] = w8[kt * 128:(kt + 1) * 128]
    w8b = np.ascontiguousarray(w8f.reshape(2, 128).T.reshape(128, 2, 1)).astype(bf16)
    bbcol = np.ascontiguousarray(bb.reshape(2, 128).T.reshape(128, 2, 1))
    battc = np.full((32, 1), batt, np.float32)
    eyeb = np.eye(128, dtype=np.float32).astype(bf16)
    wbin_bf = np.ascontiguousarray(wbin.reshape(C, 2, 128)).astype(bf16)

    # e4 pattern rows (burst window = 512 cols = 4 i-rows x 128 j)
    e4rows = np.zeros((4, L * L), np.float32)
    colblk = (np.arange(L * L) // 128) % 4
    for m in range(4):
        e4rows[m, colblk == m] = 1.0

    in_maps = []
    for b in range(B):
        X = lf[b]
        binT_x = np.empty((116, L * L), bf16)
        binT_x[0:112] = bfe[b].reshape(L * L, C).T.astype(bf16)
        binT_x[112:116] = e4rows.astype(bf16)
        xw = np.ascontiguousarray(np.concatenate([
            X.T.reshape(2, 128, N).transpose(1, 0, 2),
            wap.reshape(2, 128, H).transpose(1, 0, 2)], axis=2))
        in_maps.append({
            "binT": binT_x,
            "xw": xw,
            "xbf0": X[0:128].astype(bf16),
            "xbf1": X[128:192].astype(bf16),
            "w4dm": w4dm,
            "w8b": w8b,
            "bbcol": bbcol,
            "battc": battc,
            "eyeb": eyeb,
            "wbin": wbin_bf,
        })
    return in_maps


def run_full(inputs, trace=False):
    from concourse.bass_utils import run_bass_kernel_spmd

    if "nc" not in _CACHE:
        _CACHE["nc"] = _build()
    nc = _CACHE["nc"]
    in_maps = _prep_inputs(
        inputs["local_feats"], inputs["binary_feats"], inputs["W_apair"],
        inputs["W_bin"], inputs["b_bin"], inputs["w_att"], inputs["b_att"],
    )
    res = run_bass_kernel_spmd(nc, in_maps, list(range(B)), trace=trace)
    out = np.stack([np.asarray(res.results[c]["out"], np.float32) for c in range(B)])
    return out, res


def kernel(**inputs):
    out, _ = run_full(inputs, trace=False)
    return out


# revision 33
# speedup vs baseline: 1.3553x; 1.0115x over previous
"""Trainium2 Bass kernel for nn_Attention_18726057410699 (gnn_message_passing).

Math (per sample b):
  Y        = X @ W_apair                                  # [192, 256]
  z[i,j,k] = Y[i,k] + Y[j,k] + (binv[i,j,k] + bb[k] if i<128 and j<128)
  s[i,j]   = sigmoid( sum_k relu(z[i,j,k]) * watt[k] + batt )
  out[i,h] = sum_j s[i,j] * X[j,h]

Key structure (all per core; data-parallel over batch B=8 -> 8 cores):
  * binT_x [116, 16384] bf16 (host-staged): binT rows + 4 one-hot rows so a
    single matmul per (burst, kt) yields binv + Y[i] + bb in psum (the Y/bb
    values sit in stationary rows 112..115, fed by an on-device transpose
    via a DRAM scratch roundtrip).
  * max-trick: relu(p + Y[j]) = max(p, -Y[j]) + Y[j]; the sum_k w*Y[j] term
    is added as a rank-1 matmul into the score psum. One fused elementwise
    op per psum tile instead of add+relu.
  * scores via fp8 DoubleRow matmuls (0.5 cyc/row, both k-tiles at once),
    watt pre-scaled x8 on host; sigmoid applies scale=1/8.
  * hi rows (i>=128, no binv): u = relu(Y[j]+Y[i]) via fused tensor_scalar /
    activation, spread across DVE/Act/Pool.
  * S^T assembled with on-chip PE transposes (no DRAM scratch for scores).
"""

import numpy as np
import ml_dtypes

B, N, H, L, C = 8, 192, 256, 128, 112
NBLO, NBHI = 32, 16  # bursts of 4 i-rows

_CACHE = {}


ENGINE_SEM = {
    "EngineType.PE": "PE_",
    "EngineType.DVE": "DVE_",
    "EngineType.Activation": "Activation_",
    "EngineType.Pool": "Pool_",
    "EngineType.SP": "SP_",
}


def _fix_sync_waits(nc):
    """walrus accepts at most ONE sync-wait per compute instruction; Tile
    emits several.  Drop self waits, push overflow onto earlier same-engine
    instructions (strictly more conservative)."""
    import dataclasses
    from collections import defaultdict

    il = [i for i in nc.all_instructions()]
    streams = defaultdict(list)
    for inst in il:
        si = getattr(inst, "sync_info", None)
        if si is None:
            continue
        upd = {u.ant_name for u in si.on_update}
        eng = str(getattr(inst, "engine", None))
        self_pfx = ENGINE_SEM.get(eng)
        keep = {}
        for w in si.on_wait:
            if w.ant_name in upd:
                continue
            if self_pfx and w.ant_name.startswith(self_pfx):
                continue
            k = w.ant_name
            if k not in keep or keep[k].wait_value < w.wait_value:
                keep[k] = w
        new = list(keep.values())
        if len(new) != len(si.on_wait):
            inst.sync_info = dataclasses.replace(si, on_wait=new)
        if type(inst).__name__ in (
            "InstMatmult", "InstTensorCopy", "InstTensorTensor",
            "InstTensorScalarPtr", "InstActivation", "InstMemset",
            "InstTensorReduce", "InstTensorTensorReduce",
        ):
            streams[eng].append(inst)

    for eng, insts in streams.items():
        overflow = []
        for inst in reversed(insts):
            si = inst.sync_info
            waits = list(si.on_wait) + overflow
            ded = {}
            for w in waits:
                if w.ant_name not in ded or ded[w.ant_name].wait_value < w.wait_value:
                    ded[w.ant_name] = w
            waits = list(ded.values())
            if len(waits) <= 1:
                inst.sync_info = dataclasses.replace(si, on_wait=waits)
                overflow = []
            else:
                inst.sync_info = dataclasses.replace(si, on_wait=[waits[-1]])
                overflow = waits[:-1]
        if overflow:
            raise RuntimeError(f"{eng}: could not place {len(overflow)} waits")


def _build():
    import concourse.bass as bass
    import concourse.tile as tile
    from concourse import bacc, mybir

    f32 = mybir.dt.float32
    bf16 = mybir.dt.bfloat16
    fp8 = mybir.dt.float8e4
    ALU = mybir.AluOpType
    ACTF = mybir.ActivationFunctionType
    DR = mybir.MatmulPerfMode.DoubleRow

    nc = bacc.Bacc()

    p_binT = nc.declare_dram_parameter("binT", [116, L * L], bf16, isOutput=False)
    p_xw = nc.declare_dram_parameter("xw", [128, 2, N + H], f32, isOutput=False)
    p_xbf0 = nc.declare_dram_parameter("xbf0", [128, H], bf16, isOutput=False)
    p_xbf1 = nc.declare_dram_parameter("xbf1", [64, H], bf16, isOutput=False)
    p_w4dm = nc.declare_dram_parameter("w4dm", [128, 2, 32, 32], fp8, isOutput=False)
    p_wbin = nc.declare_dram_parameter("wbin", [112, 2, 128], bf16, isOutput=False)
    p_w8b = nc.declare_dram_parameter("w8b", [128, 2, 1], bf16, isOutput=False)
    p_bb = nc.declare_dram_parameter("bbcol", [128, 2, 1], f32, isOutput=False)
    p_batt = nc.declare_dram_parameter("battc", [32, 1], f32, isOutput=False)
    p_eye = nc.declare_dram_parameter("eyeb", [128, 128], bf16, isOutput=False)
    p_out = nc.declare_dram_parameter("out", [N, H], f32, isOutput=True)

    # Pool cannot touch PSUM (walrus verifier), so lo units are DVE max-trick
    # or Act-path (PE folds y_j into psum, Act does relu+fp8 from psum).
    # Slot = (group g 0..7, kt): Act-path slots chosen to balance engines.
    ACT_SLOTS = {(g, 1) for g in range(1, 8)}  # 7 slots = 28 kt-half units
    # hi op engine per (kt*4 + m) within a burst: Pool (all-SBUF legal)
    HI_ENG = ["P", "P", "P", "P", "P", "P", "P", "P"]

    with tile.TileContext(nc) as tc:
        with (
            tc.tile_pool(name="const", bufs=1) as cpool,
            tc.tile_pool(name="ulo", bufs=3) as ulopool,
            tc.tile_pool(name="uhi", bufs=3) as uhipool,
            tc.tile_pool(name="pbin0", bufs=2, space=bass.MemorySpace.PSUM) as pb0pool,
            tc.tile_pool(name="pbin1", bufs=2, space=bass.MemorySpace.PSUM) as pb1pool,
            tc.tile_pool(name="pscore", bufs=2, space=bass.MemorySpace.PSUM) as pspool,
            tc.tile_pool(name="pmisc", bufs=1, space=bass.MemorySpace.PSUM) as pmpool,
            tc.tile_pool(name="dram", bufs=1, space=bass.MemorySpace.DRAM) as dpool,
        ):
            # ---------------- param loads ----------------
            binT_x = cpool.tile([116, L * L], bf16, tag="binTx")
            CH = 2048
            for ch in range(8):
                nc.sync.dma_start(binT_x[:, ch * CH:(ch + 1) * CH],
                                  p_binT[:, ch * CH:(ch + 1) * CH])

            bc_sb = cpool.tile([32, 1], f32, tag="battc")
            nc.scalar.dma_start(bc_sb[:, :], p_batt[:, :])
            xw_sb = cpool.tile([128, 2, N + H], f32, tag="xw")
            nc.scalar.dma_start(xw_sb[:, :, :], p_xw[:, :, :])
            eye_b = cpool.tile([128, 128], bf16, tag="eyeb")
            nc.scalar.dma_start(eye_b[:, :], p_eye[:, :])
            w4dm = cpool.tile([128, 2, 32, 32], fp8, tag="w4dm")
            nc.scalar.dma_start(w4dm[:, :, :, :], p_w4dm[:, :, :, :])
            # warm the sigmoid table first so Copy/Relu/Sigmoid share one load
            sigwarm = cpool.tile([1, 1], bf16, tag="sigwarm")
            nc.scalar.activation(sigwarm[:, :], bc_sb[0:1, 0:1], ACTF.Sigmoid, bias=0.0, scale=1.0)

            wbin_bf = cpool.tile([112, 2, 128], bf16, tag="wbin")
            nc.gpsimd.dma_start(wbin_bf[:, :, :], p_wbin[:, :, :])
            x_bf0 = cpool.tile([128, H], bf16, tag="xbf0")
            nc.gpsimd.dma_start(x_bf0[:, :], p_xbf0[:, :])
            x_bf1 = cpool.tile([64, H], bf16, tag="xbf1")
            nc.gpsimd.dma_start(x_bf1[:, :], p_xbf1[:, :])
            w8b_sb = cpool.tile([128, 2, 1], bf16, tag="w8b")
            nc.gpsimd.dma_start(w8b_sb[:, :, :], p_w8b[:, :, :])
            bb_sb = cpool.tile([128, 2, 1], f32, tag="bbcol")
            nc.gpsimd.dma_start(bb_sb[:, :, :], p_bb[:, :, :])

            # ---------------- Y and derived tiles ----------------
            yt_f = cpool.tile([128, 2, N], f32, tag="ytf")
            yt_b = cpool.tile([128, 2, N], bf16, tag="ytb")
            ytbb = cpool.tile([128, 2, 128], bf16, tag="ytbb")
            ytN = cpool.tile([128, 2, 128], bf16, tag="ytN")
            for kt in range(2):
                psy = pmpool.tile([128, H], f32, tag="po")
                for ht in range(2):
                    nc.tensor.matmul(
                        psy[:, 0:N],
                        xw_sb[:, ht, N + kt * 128:N + (kt + 1) * 128],
                        xw_sb[:, ht, 0:N],
                        start=(ht == 0), stop=(ht == 1),
                    )
                nc.vector.tensor_copy(yt_f[:, kt, :], psy[:, 0:N])
                nc.scalar.copy(yt_b[:, kt, :], psy[:, 0:N])
            for kt in range(2):
                nc.vector.tensor_scalar(
                    ytbb[:, kt, :], yt_b[:, kt, 0:128], bb_sb[:, kt, 0:1], None, ALU.add)
            nc.vector.tensor_scalar(
                ytN[:, :, :], yt_b[:, :, 0:128], -1.0, None, ALU.mult)

            # stat_all: [128, 32, 2, 128]; rows 0..111 wbin replicated (Pool),
            # rows 112..115 Ypb rows per burst (via DRAM scratch roundtrip).
            stat_all = cpool.tile([128, NBLO, 2, 128], bf16, tag="statall")
            nc.gpsimd.tensor_copy(stat_all[0:112, 0, :, :], wbin_bf[:, :, :])
            g = 1
            while g < NBLO:
                n = min(g, NBLO - g)
                nc.gpsimd.tensor_copy(stat_all[0:112, g:g + n, :, :],
                                      stat_all[0:112, 0:n, :, :])
                g += n

            scr4 = dpool.tile([4, NBLO, 2, 128], bf16, tag="scr4")
            ypb0 = cpool.tile([128, 2, 128], bf16, tag="ypb0")
            yn0 = cpool.tile([128, 2, 128], bf16, tag="yn0")
            for kt in range(2):
                pT = pmpool.tile([128, 128], bf16, tag="pT")
                nc.tensor.transpose(pT[:, :], ytbb[:, kt, :], eye_b[:, :])
                nc.vector.tensor_copy(ypb0[:, kt, :], pT[:, :])
                # scr4[m, ib, kt, :] = Ypb[4*ib + m, kt-half]; ypb0 row j=4ib+m
                dst = scr4[:, :, kt, :].transpose([1, 0, 2])  # iterate (ib, m, k)
                nc.scalar.dma_start(dst, ypb0[:, kt, :])
                # Y natural (no bb) for the Act-path y_j fold
                pT2s = pmpool.tile([128, 128], bf16, tag="pT")
                nc.tensor.transpose(pT2s[:, :], yt_b[:, kt, 0:128], eye_b[:, :])
                nc.vector.tensor_copy(yn0[:, kt, :], pT2s[:, :])
            nc.scalar.dma_start(stat_all[112:116, :, :, :], scr4[:, :, :, :])

            # eyer4[j', m, j] = eye[j', j] for all m (Act-path fold moving)
            eyer4 = cpool.tile([128, 4, 128], bf16, tag="eyer4")
            nc.vector.tensor_copy(
                eyer4[:, :, :], eye_b[:, :].unsqueeze(1).broadcast_to([128, 4, 128]))

            # wb8h[kt] = sum_{k in kt half} w8 * Y^T[k, j] (per-half correction)
            # ones32 applies to all 32 group rows; ind0/ind1 to one 16-row half
            ones32 = cpool.tile([1, 32], bf16, tag="ones32")
            nc.vector.memset(ones32[:, :], 1.0)
            ind0 = cpool.tile([1, 32], bf16, tag="ind0")
            nc.vector.memset(ind0[:, :], 0.0)
            nc.vector.memset(ind0[:, 0:16], 1.0)
            ind1 = cpool.tile([1, 32], bf16, tag="ind1")
            nc.vector.memset(ind1[:, :], 0.0)
            nc.vector.memset(ind1[:, 16:32], 1.0)
            wb8 = cpool.tile([1, 2, 128], bf16, tag="wb8")
            pw_t = pmpool.tile([128, H], f32, tag="po")
            for kt in range(2):
                pw = pw_t[0:1, kt * 128:kt * 128 + 128]
                nc.tensor.matmul(pw, w8b_sb[:, kt, :], yt_b[:, kt, 0:128],
                                 start=True, stop=True)
                nc.vector.tensor_copy(wb8[:, kt, :], pw)

            lo_s = cpool.tile([128, 128], bf16, tag="los")
            hi_s = cpool.tile([64, N], bf16, tag="his")

            # ---------------- burst bodies ----------------
            lo_state = {}
            hi_state = {}

            def lo_burst(ib):
                g = ib // 4
                r0 = (ib % 4) * 4
                u = ulopool.tile([128, 2, 4, 128], fp8, tag="ulo")
                for kt in range(2):
                    act_path = (g, kt) in ACT_SLOTS
                    pool = pb0pool if kt == 0 else pb1pool
                    pb = pool.tile([128, 4, 128], f32, tag=f"pb{kt}")
                    nc.tensor.matmul(
                        pb[:, :, :],
                        stat_all[0:116, ib, kt, :],
                        binT_x[0:116, ib * 512:(ib + 1) * 512],
                        start=True, stop=not act_path,
                    )
                    if act_path:
                        # fold y_j into psum, then relu+fp8 on Act
                        nc.tensor.matmul(pb[:, :, :], yn0[:, kt, :], eyer4[:, :, :],
                                         start=False, stop=True)
                        nc.scalar.activation(u[:, kt, :, :], pb[:, :, :],
                                             ACTF.Relu, bias=0.0, scale=1.0)
                    else:
                        ytn_bc = ytN[:, kt, :].unsqueeze(1).broadcast_to([128, 4, 128])
                        nc.vector.tensor_tensor(u[:, kt, :, :], pb[:, :, :], ytn_bc, ALU.max)
                r0g = (ib % 8) * 4
                if ib % 8 == 0:
                    sps_t = pspool.tile([32, N], f32, tag="score")
                    lo_state["ps"] = sps_t
                sps = lo_state["ps"]
                for m in range(4):
                    nc.tensor.matmul(
                        sps[0:32, 0:128], w4dm[:, :, r0g + m, :], u[:, :, m, :],
                        start=(ib % 8 == 0 and m == 0), stop=False,
                        perf_mode=DR, skip_group_check=True,
                    )
                if ib % 8 == 7:
                    # wb correction per (kt, group-half) for max-trick halves
                    g2 = ib // 8
                    mms = []
                    for kt in range(2):
                        halves = [h for h in (2 * g2, 2 * g2 + 1)
                                  if (h, kt) not in ACT_SLOTS]
                        if len(halves) == 2:
                            mms.append((ones32, kt))
                        elif halves == [2 * g2]:
                            mms.append((ind0, kt))
                        elif halves == [2 * g2 + 1]:
                            mms.append((ind1, kt))
                    for ci, (ind, kt) in enumerate(mms):
                        nc.tensor.matmul(sps[0:32, 0:128], ind[:, :], wb8[:, kt, :],
                                         start=False, stop=(ci == len(mms) - 1),
                                         skip_group_check=True)
                    nc.scalar.activation(
                        lo_s[g2 * 32:(g2 + 1) * 32, :], sps[0:32, 0:128],
                        ACTF.Sigmoid, bias=bc_sb[0:32, 0:1], scale=0.125)

            def hi_burst(hb):
                i0 = 128 + 4 * hb
                r0g = (hb % 8) * 4
                u = uhipool.tile([128, 2, 4, N], fp8, tag="uhi")
                for kt in range(2):
                    for m in range(4):
                        i = i0 + m
                        eng = HI_ENG[kt * 4 + m]
                        if eng == "A":
                            nc.scalar.activation(
                                u[:, kt, m, :], yt_b[:, kt, :], ACTF.Relu,
                                bias=yt_f[:, kt, i:i + 1], scale=1.0)
                        elif eng == "D":
                            nc.vector.tensor_scalar(
                                u[:, kt, m, :], yt_b[:, kt, :],
                                yt_f[:, kt, i:i + 1], 0.0, ALU.add, ALU.max)
                        else:
                            nc.gpsimd.tensor_scalar(
                                u[:, kt, m, :], yt_b[:, kt, :],
                                yt_f[:, kt, i:i + 1], 0.0, ALU.add, ALU.max)
                if hb % 8 == 0:
                    sph_t = pspool.tile([32, N], f32, tag="score")
                    hi_state["ps"] = sph_t
                sph = hi_state["ps"]
                for m in range(4):
                    nc.tensor.matmul(
                        sph[0:32, 0:N], w4dm[:, :, r0g + m, :], u[:, :, m, :],
                        start=(hb % 8 == 0 and m == 0), stop=(hb % 8 == 7 and m == 3),
                        perf_mode=DR, skip_group_check=True,
                    )
                if hb % 8 == 7:
                    gh = hb // 8
                    nc.scalar.activation(
                        hi_s[gh * 32:(gh + 1) * 32, :], sph[0:32, 0:N],
                        ACTF.Sigmoid, bias=bc_sb[0:32, 0:1], scale=0.125)

            st0 = cpool.tile([128, N], bf16, tag="st0")
            st1 = cpool.tile([64, N], bf16, tag="st1")

            # hi bursts need only yt; lo bursts wait on stat_all + binT chunks.
            hi_burst(0)
            hi_burst(1)
            hi_burst(2)
            hi_burst(3)
            hi_next = 4
            for k in range(16):
                lo_burst(2 * k)
                lo_burst(2 * k + 1)
                if 2 * k + 1 == 15:
                    # lo groups 0,1 complete: transpose lo_s rows 0..63
                    pT1a_t = pmpool.tile([128, 128], bf16, tag="pT")
                    pT1a = pT1a_t[:, 0:64]
                    nc.tensor.transpose(pT1a, lo_s[0:64, :], eye_b[0:64, 0:64])
                    nc.vector.tensor_copy(st0[:, 0:64], pT1a)
                if hi_next < NBHI:
                    hi_burst(hi_next)
                    hi_next += 1
                    if hi_next == NBHI:
                        # hi complete: everything hi-dependent can go now
                        pT2_t = pmpool.tile([128, 128], bf16, tag="pT")
                        pT2 = pT2_t[:, 0:64]
                        nc.tensor.transpose(pT2, hi_s[:, 0:128], eye_b[0:64, 0:64])
                        nc.vector.tensor_copy(st0[:, 128:N], pT2)
                        pT3_t = pmpool.tile([128, 128], bf16, tag="pT")
                        pT3 = pT3_t[0:64, 0:64]
                        nc.tensor.transpose(pT3, hi_s[:, 128:N], eye_b[0:64, 0:64])
                        nc.vector.tensor_copy(st1[:, 128:N], pT3)
                        nc.vector.tensor_copy(st1[:, 0:128], hi_s[:, 0:128])

            # ---------------- finale ----------------
            pT1b_t = pmpool.tile([128, 128], bf16, tag="pT")
            pT1b = pT1b_t[:, 0:64]
            nc.tensor.transpose(pT1b, lo_s[64:128, :], eye_b[64:128, 64:128])
            nc.vector.tensor_copy(st0[:, 64:128], pT1b)

            for it, (lo, sz) in enumerate(((0, 128), (128, 64))):
                po = pmpool.tile([128, H], f32, tag="po")
                nc.tensor.matmul(po[0:sz, :], st1[:, lo:lo + sz], x_bf1[:, :],
                                 start=True, stop=False)
                nc.tensor.matmul(po[0:sz, :], st0[:, lo:lo + sz], x_bf0[:, :],
                                 start=False, stop=True)
                ob = cpool.tile([sz, H], f32, tag=f"ob{it}")
                nc.vector.tensor_copy(ob[:, :], po[0:sz, :])
                nc.sync.dma_start(p_out[lo:lo + sz, :], ob[:, :])

    nc.compile()
    return nc


def _prep_inputs(local_feats, binary_feats, W_apair, W_bin, b_bin, w_att, b_att):
    bf16 = ml_dtypes.bfloat16
    fp8 = ml_dtypes.float8_e4m3
    lf = np.asarray(local_feats, np.float32)
    bfe = np.asarray(binary_feats, np.float32)
    wap = np.ascontiguousarray(np.asarray(W_apair, np.float32))
    wbin = np.asarray(W_bin, np.float32)
    bb = np.asarray(b_bin, np.float32).reshape(-1)
    wa = np.asarray(w_att, np.float32).reshape(-1)
    batt = np.float32(np.asarray(b_att).reshape(-1)[0])

    w8 = (8.0 * wa).astype(fp8)                      # [256] fp8
    w8f = w8.astype(np.float32)
    # w4dm[k, kt, c_sel, c] = w8[kt*128+k] iff c == c_sel
    w4dm = np.zeros((128, 2, 32, 32), fp8)
    for kt in range(2):
        for c in range(32):
            w4dm[:, kt, c, c] = w8[kt * 128:(kt + 1) * 128]
    w8b = np.ascontiguousarray(w8f.reshape(2, 128).T.reshape(128, 2, 1)).astype(bf16)
    bbcol = np.ascontiguousarray(bb.reshape(2, 128).T.reshape(128, 2, 1))
    battc = np.full((32, 1), batt, np.float32)
    eyeb = np.eye(128, dtype=np.float32).astype(bf16)
    wbin_bf = np.ascontiguousarray(wbin.reshape(C, 2, 128)).astype(bf16)

    # e4 pattern rows (burst window = 512 cols = 4 i-rows x 128 j)
    e4rows = np.zeros((4, L * L), np.float32)
    colblk = (np.arange(L * L) // 128) % 4
    for m in range(4):
        e4rows[m, colblk == m] = 1.0

    in_maps = []
    for b in range(B):
        X = lf[b]
        binT_x = np.empty((116, L * L), bf16)
        binT_x[0:112] = bfe[b].reshape(L * L, C).T.astype(bf16)
        binT_x[112:116] = e4rows.astype(bf16)
        xw = np.ascontiguousarray(np.concatenate([
            X.T.reshape(2, 128, N).transpose(1, 0, 2),
            wap.reshape(2, 128, H).transpose(1, 0, 2)], axis=2))
        in_maps.append({
            "binT": binT_x,
            "xw": xw,
            "xbf0": X[0:128].astype(bf16),
            "xbf1": X[128:192].astype(bf16),
            "w4dm": w4dm,
            "w8b": w8b,
            "bbcol": bbcol,
            "battc": battc,
            "eyeb": eyeb,
            "wbin": wbin_bf,
        })
    return in_maps


def run_full(inputs, trace=False):
    from concourse.bass_utils import run_bass_kernel_spmd

    if "nc" not in _CACHE:
        _CACHE["nc"] = _build()
    nc = _CACHE["nc"]
    in_maps = _prep_inputs(
        inputs["local_feats"], inputs["binary_feats"], inputs["W_apair"],
        inputs["W_bin"], inputs["b_bin"], inputs["w_att"], inputs["b_att"],
    )
    res = run_bass_kernel_spmd(nc, in_maps, list(range(B)), trace=trace)
    out = np.stack([np.asarray(res.results[c]["out"], np.float32) for c in range(B)])
    return out, res


def kernel(**inputs):
    out, _ = run_full(inputs, trace=False)
    return out


# revision 39
# speedup vs baseline: 1.4791x; 1.0914x over previous
"""Trainium2 Bass kernel for nn_Attention_18726057410699 (gnn_message_passing).

Math (per sample b):
  Y        = X @ W_apair                                  # [192, 256]
  z[i,j,k] = Y[i,k] + Y[j,k] + (binv[i,j,k] + bb[k] if i<128 and j<128)
  s[i,j]   = sigmoid( sum_k relu(z[i,j,k]) * watt[k] + batt )
  out[i,h] = sum_j s[i,j] * X[j,h]

Key structure (all per core; data-parallel over batch B=8 -> 8 cores):
  * binT_x [116, 16384] bf16 (host-staged): binT rows + 4 one-hot rows so a
    single matmul per (burst, kt) yields binv + Y[i] + bb in psum (the Y/bb
    values sit in stationary rows 112..115, fed by an on-device transpose
    via a DRAM scratch roundtrip).
  * max-trick: relu(p + Y[j]) = max(p, -Y[j]) + Y[j]; the sum_k w*Y[j] term
    is added as a rank-1 matmul into the score psum. One fused elementwise
    op per psum tile instead of add+relu.
  * scores via fp8 DoubleRow matmuls (0.5 cyc/row, both k-tiles at once),
    watt pre-scaled x8 on host; sigmoid applies scale=1/8.
  * hi rows (i>=128, no binv): u = relu(Y[j]+Y[i]) via fused tensor_scalar /
    activation, spread across DVE/Act/Pool.
  * S^T assembled with on-chip PE transposes (no DRAM scratch for scores).
"""

import numpy as np
import ml_dtypes

B, N, H, L, C = 8, 192, 256, 128, 112
NBLO, NBHI = 32, 16  # bursts of 4 i-rows

_CACHE = {}


ENGINE_SEM = {
    "EngineType.PE": "PE_",
    "EngineType.DVE": "DVE_",
    "EngineType.Activation": "Activation_",
    "EngineType.Pool": "Pool_",
    "EngineType.SP": "SP_",
}


def _fix_sync_waits(nc):
    """walrus accepts at most ONE sync-wait per compute instruction; Tile
    emits several.  Drop self waits, push overflow onto earlier same-engine
    instructions (strictly more conservative)."""
    import dataclasses
    from collections import defaultdict

    il = [i for i in nc.all_instructions()]
    streams = defaultdict(list)
    for inst in il:
        si = getattr(inst, "sync_info", None)
        if si is None:
            continue
        upd = {u.ant_name for u in si.on_update}
        eng = str(getattr(inst, "engine", None))
        self_pfx = ENGINE_SEM.get(eng)
        keep = {}
        for w in si.on_wait:
            if w.ant_name in upd:
                continue
            if self_pfx and w.ant_name.startswith(self_pfx):
                continue
            k = w.ant_name
            if k not in keep or keep[k].wait_value < w.wait_value:
                keep[k] = w
        new = list(keep.values())
        if len(new) != len(si.on_wait):
            inst.sync_info = dataclasses.replace(si, on_wait=new)
        if type(inst).__name__ in (
            "InstMatmult", "InstTensorCopy", "InstTensorTensor",
            "InstTensorScalarPtr", "InstActivation", "InstMemset",
            "InstTensorReduce", "InstTensorTensorReduce",
        ):
            streams[eng].append(inst)

    for eng, insts in streams.items():
        overflow = []
        for inst in reversed(insts):
            si = inst.sync_info
            waits = list(si.on_wait) + overflow
            ded = {}
            for w in waits:
                if w.ant_name not in ded or ded[w.ant_name].wait_value < w.wait_value:
                    ded[w.ant_name] = w
            waits = list(ded.values())
            if len(waits) <= 1:
                inst.sync_info = dataclasses.replace(si, on_wait=waits)
                overflow = []
            else:
                inst.sync_info = dataclasses.replace(si, on_wait=[waits[-1]])
                overflow = waits[:-1]
        if overflow:
            raise RuntimeError(f"{eng}: could not place {len(overflow)} waits")


def _build():
    import concourse.bass as bass
    import concourse.tile as tile
    from concourse import bacc, mybir

    f32 = mybir.dt.float32
    bf16 = mybir.dt.bfloat16
    fp8 = mybir.dt.float8e4
    ALU = mybir.AluOpType
    ACTF = mybir.ActivationFunctionType
    DR = mybir.MatmulPerfMode.DoubleRow

    nc = bacc.Bacc()

    p_binT = nc.declare_dram_parameter("binT", [116, L * L], bf16, isOutput=False)
    p_xw = nc.declare_dram_parameter("xw", [128, 2, N + H], f32, isOutput=False)
    p_xbf0 = nc.declare_dram_parameter("xbf0", [128, H], bf16, isOutput=False)
    p_xbf1 = nc.declare_dram_parameter("xbf1", [64, H], bf16, isOutput=False)
    p_w4dm = nc.declare_dram_parameter("w4dm", [128, 2, 32, 32], fp8, isOutput=False)
    p_wbin32 = nc.declare_dram_parameter("wbin32", [112, NBLO, 2, 128], bf16, isOutput=False)
    p_w8b = nc.declare_dram_parameter("w8b", [128, 2, 1], bf16, isOutput=False)
    p_bb = nc.declare_dram_parameter("bbcol", [128, 2, 1], f32, isOutput=False)
    p_batt = nc.declare_dram_parameter("battc", [32, 1], f32, isOutput=False)
    p_eye = nc.declare_dram_parameter("eyeb", [128, 128], bf16, isOutput=False)
    p_out = nc.declare_dram_parameter("out", [N, H], f32, isOutput=True)

    # Pool cannot touch PSUM (walrus verifier), so lo units are DVE max-trick
    # or Act-path (PE folds y_j into psum, Act does relu+fp8 from psum).
    # Slot = (group g 0..7, kt): Act-path slots chosen to balance engines.
    ACT_SLOTS = {(g, 1) for g in range(1, 8)}  # 7 slots = 28 kt-half units
    # hi op engine per (kt*4 + m) within a burst: Pool (all-SBUF legal)
    HI_ENG = ["P", "P", "P", "P", "P", "P", "P", "P"]

    with tile.TileContext(nc) as tc:
        with (
            tc.tile_pool(name="const", bufs=1) as cpool,
            tc.tile_pool(name="ulo", bufs=3) as ulopool,
            tc.tile_pool(name="uhi", bufs=3) as uhipool,
            tc.tile_pool(name="pbin0", bufs=2, space=bass.MemorySpace.PSUM) as pb0pool,
            tc.tile_pool(name="pbin1", bufs=2, space=bass.MemorySpace.PSUM) as pb1pool,
            tc.tile_pool(name="pscore", bufs=2, space=bass.MemorySpace.PSUM) as pspool,
            tc.tile_pool(name="pmisc", bufs=1, space=bass.MemorySpace.PSUM) as pmpool,
            tc.tile_pool(name="dram", bufs=1, space=bass.MemorySpace.DRAM) as dpool,
        ):
            # ---------------- param loads ----------------
            # SP queue: xw first (gates Y), then binT/wbin32 interleaved in
            # need order, x_bf at the end (finale only).
            xw_sb = cpool.tile([128, 2, N + H], f32, tag="xw")
            nc.sync.dma_start(xw_sb[:, :, :], p_xw[:, :, :])
            binT_x = cpool.tile([116, L * L], bf16, tag="binTx")
            stat_all = cpool.tile([128, NBLO, 2, 128], bf16, tag="statall")
            CH = 2048
            ch_order = [("b", 0), ("w", 0), ("b", 1), ("b", 2), ("w", 1), ("b", 3),
                        ("b", 4), ("w", 2), ("b", 5), ("b", 6), ("w", 3), ("b", 7)]
            for kind, c in ch_order:
                if kind == "b":
                    nc.sync.dma_start(binT_x[:, c * CH:(c + 1) * CH],
                                      p_binT[:, c * CH:(c + 1) * CH])
                else:
                    nc.sync.dma_start(stat_all[0:112, c * 8:(c + 1) * 8, :, :],
                                      p_wbin32[:, c * 8:(c + 1) * 8, :, :])
            x_bf0 = cpool.tile([128, H], bf16, tag="xbf0")
            nc.sync.dma_start(x_bf0[:, :], p_xbf0[:, :])
            x_bf1 = cpool.tile([64, H], bf16, tag="xbf1")
            nc.sync.dma_start(x_bf1[:, :], p_xbf1[:, :])

            bc_sb = cpool.tile([32, 1], f32, tag="battc")
            nc.scalar.dma_start(bc_sb[:, :], p_batt[:, :])
            eye_b = cpool.tile([128, 128], bf16, tag="eyeb")
            nc.scalar.dma_start(eye_b[:, :], p_eye[:, :])
            # warm the sigmoid table first so Copy/Relu/Sigmoid share one load
            sigwarm = cpool.tile([1, 1], bf16, tag="sigwarm")
            nc.scalar.activation(sigwarm[:, :], bc_sb[0:1, 0:1], ACTF.Sigmoid, bias=0.0, scale=1.0)

            w4dm = cpool.tile([128, 2, 32, 32], fp8, tag="w4dm")
            nc.gpsimd.dma_start(w4dm[:, :, :, :], p_w4dm[:, :, :, :])
            w8b_sb = cpool.tile([128, 2, 1], bf16, tag="w8b")
            nc.gpsimd.dma_start(w8b_sb[:, :, :], p_w8b[:, :, :])
            bb_sb = cpool.tile([128, 2, 1], f32, tag="bbcol")
            nc.gpsimd.dma_start(bb_sb[:, :, :], p_bb[:, :, :])

            # ---------------- Y and derived tiles ----------------
            yt_f = cpool.tile([128, 2, N], f32, tag="ytf")
            yt_b = cpool.tile([128, 2, N], bf16, tag="ytb")
            ytbb = cpool.tile([128, 2, 128], bf16, tag="ytbb")
            ytN = cpool.tile([128, 2, 128], bf16, tag="ytN")
            for kt in range(2):
                psy = pmpool.tile([128, H], f32, tag="po")
                for ht in range(2):
                    nc.tensor.matmul(
                        psy[:, 0:N],
                        xw_sb[:, ht, N + kt * 128:N + (kt + 1) * 128],
                        xw_sb[:, ht, 0:N],
                        start=(ht == 0), stop=(ht == 1),
                    )
                nc.vector.tensor_copy(yt_f[:, kt, :], psy[:, 0:N])
                nc.scalar.copy(yt_b[:, kt, :], psy[:, 0:N])
            for kt in range(2):
                nc.vector.tensor_scalar(
                    ytbb[:, kt, :], yt_b[:, kt, 0:128], bb_sb[:, kt, 0:1], None, ALU.add)
            nc.vector.tensor_scalar(
                ytN[:, :, :], yt_b[:, :, 0:128], -1.0, None, ALU.mult)

            # stat_all rows 0..111 (wbin replicated) arrive via the wbin32
            # DMAs above; rows 112..115 below via the DRAM scratch roundtrip.

            scr4 = dpool.tile([4, NBLO, 2, 128], bf16, tag="scr4")
            ypb0 = cpool.tile([128, 2, 128], bf16, tag="ypb0")
            yn0 = cpool.tile([128, 2, 128], bf16, tag="yn0")
            for kt in range(2):
                pT = pmpool.tile([128, 128], bf16, tag="pT")
                nc.tensor.transpose(pT[:, :], ytbb[:, kt, :], eye_b[:, :])
                nc.vector.tensor_copy(ypb0[:, kt, :], pT[:, :])
                # scr4[m, ib, kt, :] = Ypb[4*ib + m, kt-half]; ypb0 row j=4ib+m
                dst = scr4[:, :, kt, :].transpose([1, 0, 2])  # iterate (ib, m, k)
                nc.scalar.dma_start(dst, ypb0[:, kt, :])
                # Y natural (no bb) for the Act-path y_j fold
                pT2s = pmpool.tile([128, 128], bf16, tag="pT")
                nc.tensor.transpose(pT2s[:, :], yt_b[:, kt, 0:128], eye_b[:, :])
                nc.vector.tensor_copy(yn0[:, kt, :], pT2s[:, :])
            # 2KB descriptors so the transfer spreads over the DMA engines
            g_dst = stat_all[112:116, :, :, :].rearrange(
                "p (x y) b c -> p x (y b c)", x=8)
            g_src = scr4[:, :, :, :].rearrange("p (x y) b c -> p x (y b c)", x=8)
            nc.scalar.dma_start(g_dst, g_src)

            # eyer4[j', m, j] = eye[j', j] for all m (Act-path fold moving)
            eyer4 = cpool.tile([128, 4, 128], bf16, tag="eyer4")
            nc.vector.tensor_copy(
                eyer4[:, :, :], eye_b[:, :].unsqueeze(1).broadcast_to([128, 4, 128]))

            # wb8h[kt] = sum_{k in kt half} w8 * Y^T[k, j] (per-half correction)
            # ones32 applies to all 32 group rows; ind0/ind1 to one 16-row half
            ones32 = cpool.tile([1, 32], bf16, tag="ones32")
            nc.vector.memset(ones32[:, :], 1.0)
            ind0 = cpool.tile([1, 32], bf16, tag="ind0")
            nc.vector.memset(ind0[:, :], 0.0)
            nc.vector.memset(ind0[:, 0:16], 1.0)
            ind1 = cpool.tile([1, 32], bf16, tag="ind1")
            nc.vector.memset(ind1[:, :], 0.0)
            nc.vector.memset(ind1[:, 16:32], 1.0)
            wb8 = cpool.tile([1, 2, 128], bf16, tag="wb8")
            pw_t = pmpool.tile([128, H], f32, tag="po")
            for kt in range(2):
                pw = pw_t[0:1, kt * 128:kt * 128 + 128]
                nc.tensor.matmul(pw, w8b_sb[:, kt, :], yt_b[:, kt, 0:128],
                                 start=True, stop=True)
                nc.vector.tensor_copy(wb8[:, kt, :], pw)

            lo_s = cpool.tile([128, 128], bf16, tag="los")
            hi_s = cpool.tile([64, N], bf16, tag="his")

            # ---------------- burst bodies ----------------
            lo_state = {}
            hi_state = {}

            def lo_burst(ib):
                g = ib // 4
                r0 = (ib % 4) * 4
                u = ulopool.tile([128, 2, 4, 128], fp8, tag="ulo")
                for kt in range(2):
                    act_path = (g, kt) in ACT_SLOTS
                    pool = pb0pool if kt == 0 else pb1pool
                    pb = pool.tile([128, 4, 128], f32, tag=f"pb{kt}")
                    nc.tensor.matmul(
                        pb[:, :, :],
                        stat_all[0:116, ib, kt, :],
                        binT_x[0:116, ib * 512:(ib + 1) * 512],
                        start=True, stop=not act_path,
                    )
                    if act_path:
                        # fold y_j into psum, then relu+fp8 on Act
                        nc.tensor.matmul(pb[:, :, :], yn0[:, kt, :], eyer4[:, :, :],
                                         start=False, stop=True)
                        nc.scalar.activation(u[:, kt, :, :], pb[:, :, :],
                                             ACTF.Relu, bias=0.0, scale=1.0)
                    else:
                        ytn_bc = ytN[:, kt, :].unsqueeze(1).broadcast_to([128, 4, 128])
                        nc.vector.tensor_tensor(u[:, kt, :, :], pb[:, :, :], ytn_bc, ALU.max)
                r0g = (ib % 8) * 4
                if ib % 8 == 0:
                    sps_t = pspool.tile([32, N], f32, tag="score")
                    lo_state["ps"] = sps_t
                sps = lo_state["ps"]
                for m in range(4):
                    nc.tensor.matmul(
                        sps[0:32, 0:128], w4dm[:, :, r0g + m, :], u[:, :, m, :],
                        start=(ib % 8 == 0 and m == 0), stop=False,
                        perf_mode=DR, skip_group_check=True,
                    )
                if ib % 8 == 7:
                    # wb correction per (kt, group-half) for max-trick halves
                    g2 = ib // 8
                    mms = []
                    for kt in range(2):
                        halves = [h for h in (2 * g2, 2 * g2 + 1)
                                  if (h, kt) not in ACT_SLOTS]
                        if len(halves) == 2:
                            mms.append((ones32, kt))
                        elif halves == [2 * g2]:
                            mms.append((ind0, kt))
                        elif halves == [2 * g2 + 1]:
                            mms.append((ind1, kt))
                    for ci, (ind, kt) in enumerate(mms):
                        nc.tensor.matmul(sps[0:32, 0:128], ind[:, :], wb8[:, kt, :],
                                         start=False, stop=(ci == len(mms) - 1),
                                         skip_group_check=True)
                    nc.scalar.activation(
                        lo_s[g2 * 32:(g2 + 1) * 32, :], sps[0:32, 0:128],
                        ACTF.Sigmoid, bias=bc_sb[0:32, 0:1], scale=0.125)

            def hi_burst(hb):
                i0 = 128 + 4 * hb
                r0g = (hb % 8) * 4
                u = uhipool.tile([128, 2, 4, N], fp8, tag="uhi")
                for kt in range(2):
                    for m in range(4):
                        i = i0 + m
                        eng = HI_ENG[kt * 4 + m]
                        if eng == "A":
                            nc.scalar.activation(
                                u[:, kt, m, :], yt_b[:, kt, :], ACTF.Relu,
                                bias=yt_f[:, kt, i:i + 1], scale=1.0)
                        elif eng == "D":
                            nc.vector.tensor_scalar(
                                u[:, kt, m, :], yt_b[:, kt, :],
                                yt_f[:, kt, i:i + 1], 0.0, ALU.add, ALU.max)
                        else:
                            nc.gpsimd.tensor_scalar(
                                u[:, kt, m, :], yt_b[:, kt, :],
                                yt_f[:, kt, i:i + 1], 0.0, ALU.add, ALU.max)
                if hb % 8 == 0:
                    sph_t = pspool.tile([32, N], f32, tag="score")
                    hi_state["ps"] = sph_t
                sph = hi_state["ps"]
                for m in range(4):
                    nc.tensor.matmul(
                        sph[0:32, 0:N], w4dm[:, :, r0g + m, :], u[:, :, m, :],
                        start=(hb % 8 == 0 and m == 0), stop=(hb % 8 == 7 and m == 3),
                        perf_mode=DR, skip_group_check=True,
                    )
                if hb % 8 == 7:
                    gh = hb // 8
                    nc.scalar.activation(
                        hi_s[gh * 32:(gh + 1) * 32, :], sph[0:32, 0:N],
                        ACTF.Sigmoid, bias=bc_sb[0:32, 0:1], scale=0.125)

            st0 = cpool.tile([128, N], bf16, tag="st0")
            st1 = cpool.tile([64, N], bf16, tag="st1")

            # hi bursts need only yt; lo bursts wait on stat_all + binT chunks.
            hi_burst(0)
            hi_burst(1)
            hi_burst(2)
            hi_burst(3)
            hi_next = 4
            for k in range(16):
                lo_burst(2 * k)
                lo_burst(2 * k + 1)
                if 2 * k + 1 == 15:
                    # lo groups 0,1 complete: transpose lo_s rows 0..63
                    pT1a_t = pmpool.tile([128, 128], bf16, tag="pT")
                    pT1a = pT1a_t[:, 0:64]
                    nc.tensor.transpose(pT1a, lo_s[0:64, :], eye_b[0:64, 0:64])
                    nc.vector.tensor_copy(st0[:, 0:64], pT1a)
                if hi_next < NBHI:
                    hi_burst(hi_next)
                    hi_next += 1
                    if hi_next == NBHI:
                        # hi complete: everything hi-dependent can go now
                        pT2_t = pmpool.tile([128, 128], bf16, tag="pT")
                        pT2 = pT2_t[:, 0:64]
                        nc.tensor.transpose(pT2, hi_s[:, 0:128], eye_b[0:64, 0:64])
                        nc.vector.tensor_copy(st0[:, 128:N], pT2)
                        pT3_t = pmpool.tile([128, 128], bf16, tag="pT")
                        pT3 = pT3_t[0:64, 0:64]
                        nc.tensor.transpose(pT3, hi_s[:, 128:N], eye_b[0:64, 0:64])
                        nc.vector.tensor_copy(st1[:, 128:N], pT3)
                        nc.vector.tensor_copy(st1[:, 0:128], hi_s[:, 0:128])

            # ---------------- finale ----------------
            pT1b_t = pmpool.tile([128, 128], bf16, tag="pT")
            pT1b = pT1b_t[:, 0:64]
            nc.tensor.transpose(pT1b, lo_s[64:128, :], eye_b[64:128, 64:128])
            nc.vector.tensor_copy(st0[:, 64:128], pT1b)

            for it, (lo, sz) in enumerate(((0, 128), (128, 64))):
                po = pmpool.tile([128, H], f32, tag="po")
                nc.tensor.matmul(po[0:sz, :], st1[:, lo:lo + sz], x_bf1[:, :],
                                 start=True, stop=False)
                nc.tensor.matmul(po[0:sz, :], st0[:, lo:lo + sz], x_bf0[:, :],
                                 start=False, stop=True)
                ob = cpool.tile([sz, H], f32, tag=f"ob{it}")
                nc.vector.tensor_copy(ob[:, :], po[0:sz, :])
                nc.sync.dma_start(p_out[lo:lo + sz, :], ob[:, :])

    nc.compile()
    return nc


def _prep_inputs(local_feats, binary_feats, W_apair, W_bin, b_bin, w_att, b_att):
    bf16 = ml_dtypes.bfloat16
    fp8 = ml_dtypes.float8_e4m3
    lf = np.asarray(local_feats, np.float32)
    bfe = np.asarray(binary_feats, np.float32)
    wap = np.ascontiguousarray(np.asarray(W_apair, np.float32))
    wbin = np.asarray(W_bin, np.float32)
    bb = np.asarray(b_bin, np.float32).reshape(-1)
    wa = np.asarray(w_att, np.float32).reshape(-1)
    batt = np.float32(np.asarray(b_att).reshape(-1)[0])

    w8 = (8.0 * wa).astype(fp8)                      # [256] fp8
    w8f = w8.astype(np.float32)
    # w4dm[k, kt, c_sel, c] = w8[kt*128+k] iff c == c_sel
    w4dm = np.zeros((128, 2, 32, 32), fp8)
    for kt in range(2):
        for c in range(32):
            w4dm[:, kt, c, c] = w8[kt * 128:(kt + 1) * 128]
    w8b = np.ascontiguousarray(w8f.reshape(2, 128).T.reshape(128, 2, 1)).astype(bf16)
    bbcol = np.ascontiguousarray(bb.reshape(2, 128).T.reshape(128, 2, 1))
    battc = np.full((32, 1), batt, np.float32)
    eyeb = np.eye(128, dtype=np.float32).astype(bf16)
    wbin_bf = np.ascontiguousarray(wbin.reshape(C, 2, 128)).astype(bf16)
    wbin32 = np.ascontiguousarray(
        np.broadcast_to(wbin_bf[:, None, :, :], (C, NBLO, 2, 128)))

    # e4 pattern rows (burst window = 512 cols = 4 i-rows x 128 j)
    e4rows = np.zeros((4, L * L), np.float32)
    colblk = (np.arange(L * L) // 128) % 4
    for m in range(4):
        e4rows[m, colblk == m] = 1.0

    in_maps = []
    for b in range(B):
        X = lf[b]
        binT_x = np.empty((116, L * L), bf16)
        binT_x[0:112] = bfe[b].reshape(L * L, C).T.astype(bf16)
        binT_x[112:116] = e4rows.astype(bf16)
        xw = np.ascontiguousarray(np.concatenate([
            X.T.reshape(2, 128, N).transpose(1, 0, 2),
            wap.reshape(2, 128, H).transpose(1, 0, 2)], axis=2))
        in_maps.append({
            "binT": binT_x,
            "xw": xw,
            "xbf0": X[0:128].astype(bf16),
            "xbf1": X[128:192].astype(bf16),
            "w4dm": w4dm,
            "w8b": w8b,
            "bbcol": bbcol,
            "battc": battc,
            "eyeb": eyeb,
            "wbin32": wbin32,
        })
    return in_maps


def run_full(inputs, trace=False):
    from concourse.bass_utils import run_bass_kernel_spmd

    if "nc" not in _CACHE:
        _CACHE["nc"] = _build()
    nc = _CACHE["nc"]
    in_maps = _prep_inputs(
        inputs["local_feats"], inputs["binary_feats"], inputs["W_apair"],
        inputs["W_bin"], inputs["b_bin"], inputs["w_att"], inputs["b_att"],
    )
    res = run_bass_kernel_spmd(nc, in_maps, list(range(B)), trace=trace)
    out = np.stack([np.asarray(res.results[c]["out"], np.float32) for c in range(B)])
    return out, res


def kernel(**inputs):
    out, _ = run_full(inputs, trace=False)
    return out


# revision 42
# speedup vs baseline: 1.4943x; 1.0103x over previous
"""Trainium2 Bass kernel for nn_Attention_18726057410699 (gnn_message_passing).

Math (per sample b):
  Y        = X @ W_apair                                  # [192, 256]
  z[i,j,k] = Y[i,k] + Y[j,k] + (binv[i,j,k] + bb[k] if i<128 and j<128)
  s[i,j]   = sigmoid( sum_k relu(z[i,j,k]) * watt[k] + batt )
  out[i,h] = sum_j s[i,j] * X[j,h]

Key structure (all per core; data-parallel over batch B=8 -> 8 cores):
  * binT_x [116, 16384] bf16 (host-staged): binT rows + 4 one-hot rows so a
    single matmul per (burst, kt) yields binv + Y[i] + bb in psum (the Y/bb
    values sit in stationary rows 112..115, fed by an on-device transpose
    via a DRAM scratch roundtrip).
  * max-trick: relu(p + Y[j]) = max(p, -Y[j]) + Y[j]; the sum_k w*Y[j] term
    is added as a rank-1 matmul into the score psum. One fused elementwise
    op per psum tile instead of add+relu.
  * scores via fp8 DoubleRow matmuls (0.5 cyc/row, both k-tiles at once),
    watt pre-scaled x8 on host; sigmoid applies scale=1/8.
  * hi rows (i>=128, no binv): u = relu(Y[j]+Y[i]) via fused tensor_scalar /
    activation, spread across DVE/Act/Pool.
  * S^T assembled with on-chip PE transposes (no DRAM scratch for scores).
"""

import numpy as np
import ml_dtypes

B, N, H, L, C = 8, 192, 256, 128, 112
NBLO, NBHI = 32, 16  # bursts of 4 i-rows

_CACHE = {}


ENGINE_SEM = {
    "EngineType.PE": "PE_",
    "EngineType.DVE": "DVE_",
    "EngineType.Activation": "Activation_",
    "EngineType.Pool": "Pool_",
    "EngineType.SP": "SP_",
}


def _fix_sync_waits(nc):
    """walrus accepts at most ONE sync-wait per compute instruction; Tile
    emits several.  Drop self waits, push overflow onto earlier same-engine
    instructions (strictly more conservative)."""
    import dataclasses
    from collections import defaultdict

    il = [i for i in nc.all_instructions()]
    streams = defaultdict(list)
    for inst in il:
        si = getattr(inst, "sync_info", None)
        if si is None:
            continue
        upd = {u.ant_name for u in si.on_update}
        eng = str(getattr(inst, "engine", None))
        self_pfx = ENGINE_SEM.get(eng)
        keep = {}
        for w in si.on_wait:
            if w.ant_name in upd:
                continue
            if self_pfx and w.ant_name.startswith(self_pfx):
                continue
            k = w.ant_name
            if k not in keep or keep[k].wait_value < w.wait_value:
                keep[k] = w
        new = list(keep.values())
        if len(new) != len(si.on_wait):
            inst.sync_info = dataclasses.replace(si, on_wait=new)
        if type(inst).__name__ in (
            "InstMatmult", "InstTensorCopy", "InstTensorTensor",
            "InstTensorScalarPtr", "InstActivation", "InstMemset",
            "InstTensorReduce", "InstTensorTensorReduce",
        ):
            streams[eng].append(inst)

    for eng, insts in streams.items():
        overflow = []
        for inst in reversed(insts):
            si = inst.sync_info
            waits = list(si.on_wait) + overflow
            ded = {}
            for w in waits:
                if w.ant_name not in ded or ded[w.ant_name].wait_value < w.wait_value:
                    ded[w.ant_name] = w
            waits = list(ded.values())
            if len(waits) <= 1:
                inst.sync_info = dataclasses.replace(si, on_wait=waits)
                overflow = []
            else:
                inst.sync_info = dataclasses.replace(si, on_wait=[waits[-1]])
                overflow = waits[:-1]
        if overflow:
            raise RuntimeError(f"{eng}: could not place {len(overflow)} waits")


def _build():
    import concourse.bass as bass
    import concourse.tile as tile
    from concourse import bacc, mybir

    f32 = mybir.dt.float32
    bf16 = mybir.dt.bfloat16
    fp8 = mybir.dt.float8e4
    ALU = mybir.AluOpType
    ACTF = mybir.ActivationFunctionType
    DR = mybir.MatmulPerfMode.DoubleRow

    nc = bacc.Bacc()

    p_binT = nc.declare_dram_parameter("binT", [116, L * L], fp8, isOutput=False)
    p_xw = nc.declare_dram_parameter("xw", [128, 2, N + H], f32, isOutput=False)
    p_xbf0 = nc.declare_dram_parameter("xbf0", [128, H], bf16, isOutput=False)
    p_xbf1 = nc.declare_dram_parameter("xbf1", [64, H], bf16, isOutput=False)
    p_w4dm = nc.declare_dram_parameter("w4dm", [128, 2, 32, 32], fp8, isOutput=False)
    p_wbin32 = nc.declare_dram_parameter("wbin32", [112, NBLO, 2, 128], fp8, isOutput=False)
    p_w8b = nc.declare_dram_parameter("w8b", [128, 2, 1], bf16, isOutput=False)
    p_bb = nc.declare_dram_parameter("bbcol", [128, 2, 1], f32, isOutput=False)
    p_batt = nc.declare_dram_parameter("battc", [32, 1], f32, isOutput=False)
    p_eye = nc.declare_dram_parameter("eyeb", [128, 128], bf16, isOutput=False)
    p_out = nc.declare_dram_parameter("out", [N, H], f32, isOutput=True)

    # Pool cannot touch PSUM (walrus verifier), so lo units are DVE max-trick
    # or Act-path (PE folds y_j into psum, Act does relu+fp8 from psum).
    # Slot = (group g 0..7, kt): Act-path slots chosen to balance engines.
    ACT_SLOTS = {(g, 1) for g in range(1, 8)}  # 7 slots = 28 kt-half units
    # hi op engine per (kt*4 + m) within a burst: Pool (all-SBUF legal)
    HI_ENG = ["P", "P", "P", "P", "P", "P", "P", "P"]

    with tile.TileContext(nc) as tc:
        with (
            tc.tile_pool(name="const", bufs=1) as cpool,
            tc.tile_pool(name="ulo", bufs=3) as ulopool,
            tc.tile_pool(name="uhi", bufs=3) as uhipool,
            tc.tile_pool(name="pbin0", bufs=2, space=bass.MemorySpace.PSUM) as pb0pool,
            tc.tile_pool(name="pbin1", bufs=2, space=bass.MemorySpace.PSUM) as pb1pool,
            tc.tile_pool(name="pscore", bufs=2, space=bass.MemorySpace.PSUM) as pspool,
            tc.tile_pool(name="pmisc", bufs=1, space=bass.MemorySpace.PSUM) as pmpool,
            tc.tile_pool(name="dram", bufs=1, space=bass.MemorySpace.DRAM) as dpool,
        ):
            # ---------------- param loads ----------------
            # SP queue: xw first (gates Y), then binT/wbin32 interleaved in
            # need order, x_bf at the end (finale only).
            xw_sb = cpool.tile([128, 2, N + H], f32, tag="xw")
            nc.sync.dma_start(xw_sb[:, :, :], p_xw[:, :, :])
            binT_x = cpool.tile([116, L * L], fp8, tag="binTx")
            stat_all = cpool.tile([128, NBLO, 2, 128], fp8, tag="statall")
            CH = 4096
            ch_order = [("b", 0), ("w", 0), ("b", 1), ("w", 1), ("b", 2), ("b", 3)]
            for kind, c in ch_order:
                if kind == "b":
                    nc.sync.dma_start(binT_x[:, c * CH:(c + 1) * CH],
                                      p_binT[:, c * CH:(c + 1) * CH])
                else:
                    nc.sync.dma_start(stat_all[0:112, c * 16:(c + 1) * 16, :, :],
                                      p_wbin32[:, c * 16:(c + 1) * 16, :, :])
            x_bf0 = cpool.tile([128, H], bf16, tag="xbf0")
            nc.sync.dma_start(x_bf0[:, :], p_xbf0[:, :])
            x_bf1 = cpool.tile([64, H], bf16, tag="xbf1")
            nc.sync.dma_start(x_bf1[:, :], p_xbf1[:, :])

            bc_sb = cpool.tile([32, 1], f32, tag="battc")
            nc.scalar.dma_start(bc_sb[:, :], p_batt[:, :])
            eye_b = cpool.tile([128, 128], bf16, tag="eyeb")
            nc.scalar.dma_start(eye_b[:, :], p_eye[:, :])
            # warm the sigmoid table first so Copy/Relu/Sigmoid share one load
            sigwarm = cpool.tile([1, 1], bf16, tag="sigwarm")
            nc.scalar.activation(sigwarm[:, :], bc_sb[0:1, 0:1], ACTF.Sigmoid, bias=0.0, scale=1.0)

            w4dm = cpool.tile([128, 2, 32, 32], fp8, tag="w4dm")
            nc.gpsimd.dma_start(w4dm[:, :, :, :], p_w4dm[:, :, :, :])
            w8b_sb = cpool.tile([128, 2, 1], bf16, tag="w8b")
            nc.gpsimd.dma_start(w8b_sb[:, :, :], p_w8b[:, :, :])
            bb_sb = cpool.tile([128, 2, 1], f32, tag="bbcol")
            nc.gpsimd.dma_start(bb_sb[:, :, :], p_bb[:, :, :])

            # ---------------- Y and derived tiles ----------------
            yt_f = cpool.tile([128, 2, N], f32, tag="ytf")
            yt_b = cpool.tile([128, 2, N], bf16, tag="ytb")
            ytbb = cpool.tile([128, 2, 128], bf16, tag="ytbb")
            ytN = cpool.tile([128, 2, 128], bf16, tag="ytN")
            for kt in range(2):
                psy = pmpool.tile([128, H], f32, tag="po")
                for ht in range(2):
                    nc.tensor.matmul(
                        psy[:, 0:N],
                        xw_sb[:, ht, N + kt * 128:N + (kt + 1) * 128],
                        xw_sb[:, ht, 0:N],
                        start=(ht == 0), stop=(ht == 1),
                    )
                nc.vector.tensor_copy(yt_f[:, kt, :], psy[:, 0:N])
                nc.vector.tensor_copy(yt_b[:, kt, :], psy[:, 0:N])
            for kt in range(2):
                nc.vector.tensor_scalar(
                    ytbb[:, kt, :], yt_b[:, kt, 0:128], bb_sb[:, kt, 0:1], None, ALU.add)
            nc.vector.tensor_scalar(
                ytN[:, :, :], yt_b[:, :, 0:128], -1.0, None, ALU.mult)

            # stat_all rows 0..111 (wbin replicated) arrive via the wbin32
            # DMAs above; rows 112..115 below via the DRAM scratch roundtrip.

            scr4 = dpool.tile([NBLO, 4, 2, 128], fp8, tag="scr4")
            ypb0 = cpool.tile([128, 2, 128], fp8, tag="ypb0")
            yn0 = cpool.tile([128, 2, 128], bf16, tag="yn0")
            for kt in range(2):
                pT = pmpool.tile([128, 128], bf16, tag="pT")
                nc.tensor.transpose(pT[:, :], ytbb[:, kt, :], eye_b[:, :])
                nc.vector.tensor_copy(ypb0[:, kt, :], pT[:, :])
                # scr4[ib, m, kt, :] = Ypb[4*ib + m, kt-half]; natural j order
                nc.scalar.dma_start(scr4[:, :, kt, :], ypb0[:, kt, :])
                # Y natural (no bb) for the Act-path y_j fold
                pT2s = pmpool.tile([128, 128], bf16, tag="pT")
                nc.tensor.transpose(pT2s[:, :], yt_b[:, kt, 0:128], eye_b[:, :])
                nc.vector.tensor_copy(yn0[:, kt, :], pT2s[:, :])
            # strided src -> many small descriptors (model spreads those
            # across the 16 DMA engines; few big descriptors are charged 2x)
            nc.scalar.dma_start(stat_all[112:116, :, :, :],
                                scr4[:, :, :, :].transpose([1, 0, 2, 3]))

            # eyer4[j', m, j] = eye[j', j] for all m (Act-path fold moving)
            eyer4 = cpool.tile([128, 4, 128], bf16, tag="eyer4")
            nc.vector.tensor_copy(
                eyer4[:, :, :], eye_b[:, :].unsqueeze(1).broadcast_to([128, 4, 128]))

            # wb8h[kt] = sum_{k in kt half} w8 * Y^T[k, j] (per-half correction)
            # ones32 applies to all 32 group rows; ind0/ind1 to one 16-row half
            ones32 = cpool.tile([1, 32], bf16, tag="ones32")
            nc.vector.memset(ones32[:, :], 1.0)
            ind0 = cpool.tile([1, 32], bf16, tag="ind0")
            nc.vector.memset(ind0[:, :], 0.0)
            nc.vector.memset(ind0[:, 0:16], 1.0)
            ind1 = cpool.tile([1, 32], bf16, tag="ind1")
            nc.vector.memset(ind1[:, :], 0.0)
            nc.vector.memset(ind1[:, 16:32], 1.0)
            wb8 = cpool.tile([1, 2, 128], bf16, tag="wb8")
            pw_t = pmpool.tile([128, H], f32, tag="po")
            for kt in range(2):
                pw = pw_t[0:1, kt * 128:kt * 128 + 128]
                nc.tensor.matmul(pw, w8b_sb[:, kt, :], yt_b[:, kt, 0:128],
                                 start=True, stop=True)
                nc.vector.tensor_copy(wb8[:, kt, :], pw)

            lo_s = cpool.tile([128, 128], bf16, tag="los")
            hi_s = cpool.tile([64, N], bf16, tag="his")

            # ---------------- burst bodies ----------------
            lo_state = {}
            hi_state = {}

            def lo_burst(ib):
                g = ib // 4
                r0 = (ib % 4) * 4
                u = ulopool.tile([128, 2, 4, 128], fp8, tag="ulo")
                for kt in range(2):
                    act_path = (g, kt) in ACT_SLOTS
                    pool = pb0pool if kt == 0 else pb1pool
                    pb = pool.tile([128, 4, 128], f32, tag=f"pb{kt}")
                    nc.tensor.matmul(
                        pb[:, :, :],
                        stat_all[0:116, ib, kt, :],
                        binT_x[0:116, ib * 512:(ib + 1) * 512],
                        start=True, stop=not act_path,
                    )
                    if act_path:
                        # fold y_j into psum, then relu+fp8 on Act
                        nc.tensor.matmul(pb[:, :, :], yn0[:, kt, :], eyer4[:, :, :],
                                         start=False, stop=True)
                        nc.scalar.activation(u[:, kt, :, :], pb[:, :, :],
                                             ACTF.Relu, bias=0.0, scale=1.0)
                    else:
                        ytn_bc = ytN[:, kt, :].unsqueeze(1).broadcast_to([128, 4, 128])
                        nc.vector.tensor_tensor(u[:, kt, :, :], pb[:, :, :], ytn_bc, ALU.max)
                r0g = (ib % 8) * 4
                if ib % 8 == 0:
                    sps_t = pspool.tile([32, N], f32, tag="score")
                    lo_state["ps"] = sps_t
                sps = lo_state["ps"]
                for m in range(4):
                    nc.tensor.matmul(
                        sps[0:32, 0:128], w4dm[:, :, r0g + m, :], u[:, :, m, :],
                        start=(ib % 8 == 0 and m == 0), stop=False,
                        perf_mode=DR, skip_group_check=True,
                    )
                if ib % 8 == 7:
                    # wb correction per (kt, group-half) for max-trick halves
                    g2 = ib // 8
                    mms = []
                    for kt in range(2):
                        halves = [h for h in (2 * g2, 2 * g2 + 1)
                                  if (h, kt) not in ACT_SLOTS]
                        if len(halves) == 2:
                            mms.append((ones32, kt))
                        elif halves == [2 * g2]:
                            mms.append((ind0, kt))
                        elif halves == [2 * g2 + 1]:
                            mms.append((ind1, kt))
                    for ci, (ind, kt) in enumerate(mms):
                        nc.tensor.matmul(sps[0:32, 0:128], ind[:, :], wb8[:, kt, :],
                                         start=False, stop=(ci == len(mms) - 1),
                                         skip_group_check=True)
                    nc.scalar.activation(
                        lo_s[g2 * 32:(g2 + 1) * 32, :], sps[0:32, 0:128],
                        ACTF.Sigmoid, bias=bc_sb[0:32, 0:1], scale=0.125)

            def hi_burst(hb):
                i0 = 128 + 4 * hb
                r0g = (hb % 8) * 4
                u = uhipool.tile([128, 2, 4, N], fp8, tag="uhi")
                for kt in range(2):
                    for m in range(4):
                        i = i0 + m
                        eng = HI_ENG[kt * 4 + m]
                        if eng == "A":
                            nc.scalar.activation(
                                u[:, kt, m, :], yt_b[:, kt, :], ACTF.Relu,
                                bias=yt_f[:, kt, i:i + 1], scale=1.0)
                        elif eng == "D":
                            nc.vector.tensor_scalar(
                                u[:, kt, m, :], yt_b[:, kt, :],
                                yt_f[:, kt, i:i + 1], 0.0, ALU.add, ALU.max)
                        else:
                            nc.gpsimd.tensor_scalar(
                                u[:, kt, m, :], yt_b[:, kt, :],
                                yt_f[:, kt, i:i + 1], 0.0, ALU.add, ALU.max)
                if hb % 8 == 0:
                    sph_t = pspool.tile([32, N], f32, tag="score")
                    hi_state["ps"] = sph_t
                sph = hi_state["ps"]
                for m in range(4):
                    nc.tensor.matmul(
                        sph[0:32, 0:N], w4dm[:, :, r0g + m, :], u[:, :, m, :],
                        start=(hb % 8 == 0 and m == 0), stop=(hb % 8 == 7 and m == 3),
                        perf_mode=DR, skip_group_check=True,
                    )
                if hb % 8 == 7:
                    gh = hb // 8
                    nc.scalar.activation(
                        hi_s[gh * 32:(gh + 1) * 32, :], sph[0:32, 0:N],
                        ACTF.Sigmoid, bias=bc_sb[0:32, 0:1], scale=0.125)

            st0 = cpool.tile([128, N], bf16, tag="st0")
            st1 = cpool.tile([64, N], bf16, tag="st1")

            def emit_strip(lo, tag):
                # out rows lo..lo+64 = (st0[:, lo:lo+64])^T x0 + (st1)^T x1
                po = pmpool.tile([128, H], f32, tag="po")
                nc.tensor.matmul(po[0:64, :], st1[:, lo:lo + 64], x_bf1[:, :],
                                 start=True, stop=False)
                nc.tensor.matmul(po[0:64, :], st0[:, lo:lo + 64], x_bf0[:, :],
                                 start=False, stop=True)
                ob = cpool.tile([64, H], f32, tag=f"ob{tag}")
                nc.vector.tensor_copy(ob[:, :], po[0:64, :])
                nc.sync.dma_start(p_out[lo:lo + 64, :], ob[:, :])

            # hi bursts need only yt; lo bursts wait on stat_all + binT chunks.
            hi_burst(0)
            hi_burst(1)
            hi_burst(2)
            hi_burst(3)
            hi_next = 4
            strip_a_ready = False
            for k in range(16):
                lo_burst(2 * k)
                lo_burst(2 * k + 1)
                if 2 * k + 1 == 15:
                    # lo groups 0,1 complete: transpose lo_s rows 0..63
                    pT1a_t = pmpool.tile([128, 128], bf16, tag="pT")
                    pT1a = pT1a_t[:, 0:64]
                    nc.tensor.transpose(pT1a, lo_s[0:64, :], eye_b[0:64, 0:64])
                    nc.vector.tensor_copy(st0[:, 0:64], pT1a)
                    strip_a_ready = True
                if hi_next < NBHI:
                    hi_burst(hi_next)
                    hi_next += 1
                    if hi_next == NBHI:
                        # hi complete: everything hi-dependent can go now
                        pT2_t = pmpool.tile([128, 128], bf16, tag="pT")
                        pT2 = pT2_t[:, 0:64]
                        nc.tensor.transpose(pT2, hi_s[:, 0:128], eye_b[0:64, 0:64])
                        nc.vector.tensor_copy(st0[:, 128:N], pT2)
                        pT3_t = pmpool.tile([128, 128], bf16, tag="pT")
                        pT3 = pT3_t[0:64, 0:64]
                        nc.tensor.transpose(pT3, hi_s[:, 128:N], eye_b[0:64, 0:64])
                        nc.vector.tensor_copy(st1[:, 128:N], pT3)
                        nc.vector.tensor_copy(st1[:, 0:128], hi_s[:, 0:128])
                        emit_strip(128, "h")       # out rows 128..191
                        assert strip_a_ready
                        emit_strip(0, "a")         # out rows 0..63

            # ---------------- finale: last strip (rows 64..127) ----------------
            pT1b_t = pmpool.tile([128, 128], bf16, tag="pT")
            pT1b = pT1b_t[:, 0:64]
            nc.tensor.transpose(pT1b, lo_s[64:128, :], eye_b[64:128, 64:128])
            nc.vector.tensor_copy(st0[:, 64:128], pT1b)
            emit_strip(64, "b")

    nc.compile()
    return nc


def _prep_inputs(local_feats, binary_feats, W_apair, W_bin, b_bin, w_att, b_att):
    bf16 = ml_dtypes.bfloat16
    fp8 = ml_dtypes.float8_e4m3
    lf = np.asarray(local_feats, np.float32)
    bfe = np.asarray(binary_feats, np.float32)
    wap = np.ascontiguousarray(np.asarray(W_apair, np.float32))
    wbin = np.asarray(W_bin, np.float32)
    bb = np.asarray(b_bin, np.float32).reshape(-1)
    wa = np.asarray(w_att, np.float32).reshape(-1)
    batt = np.float32(np.asarray(b_att).reshape(-1)[0])

    w8 = (8.0 * wa).astype(fp8)                      # [256] fp8
    w8f = w8.astype(np.float32)
    # w4dm[k, kt, c_sel, c] = w8[kt*128+k] iff c == c_sel
    w4dm = np.zeros((128, 2, 32, 32), fp8)
    for kt in range(2):
        for c in range(32):
            w4dm[:, kt, c, c] = w8[kt * 128:(kt + 1) * 128]
    w8b = np.ascontiguousarray(w8f.reshape(2, 128).T.reshape(128, 2, 1)).astype(bf16)
    bbcol = np.ascontiguousarray(bb.reshape(2, 128).T.reshape(128, 2, 1))
    battc = np.full((32, 1), batt, np.float32)
    eyeb = np.eye(128, dtype=np.float32).astype(bf16)
    wbin_f8 = np.ascontiguousarray(wbin.reshape(C, 2, 128)).astype(fp8)
    wbin32 = np.ascontiguousarray(
        np.broadcast_to(wbin_f8[:, None, :, :], (C, NBLO, 2, 128)))

    # e4 pattern rows (burst window = 512 cols = 4 i-rows x 128 j)
    e4rows = np.zeros((4, L * L), np.float32)
    colblk = (np.arange(L * L) // 128) % 4
    for m in range(4):
        e4rows[m, colblk == m] = 1.0

    in_maps = []
    for b in range(B):
        X = lf[b]
        binT_x = np.empty((116, L * L), fp8)
        binT_x[0:112] = bfe[b].reshape(L * L, C).T.astype(fp8)
        binT_x[112:116] = e4rows.astype(fp8)
        xw = np.ascontiguousarray(np.concatenate([
            X.T.reshape(2, 128, N).transpose(1, 0, 2),
            wap.reshape(2, 128, H).transpose(1, 0, 2)], axis=2))
        in_maps.append({
            "binT": binT_x,
            "xw": xw,
            "xbf0": X[0:128].astype(bf16),
            "xbf1": X[128:192].astype(bf16),
            "w4dm": w4dm,
            "w8b": w8b,
            "bbcol": bbcol,
            "battc": battc,
            "eyeb": eyeb,
            "wbin32": wbin32,
        })
    return in_maps


def run_full(inputs, trace=False):
    from concourse.bass_utils import run_bass_kernel_spmd

    if "nc" not in _CACHE:
        _CACHE["nc"] = _build()
    nc = _CACHE["nc"]
    in_maps = _prep_inputs(
        inputs["local_feats"], inputs["binary_feats"], inputs["W_apair"],
        inputs["W_bin"], inputs["b_bin"], inputs["w_att"], inputs["b_att"],
    )
    res = run_bass_kernel_spmd(nc, in_maps, list(range(B)), trace=trace)
    out = np.stack([np.asarray(res.results[c]["out"], np.float32) for c in range(B)])
    return out, res


def kernel(**inputs):
    out, _ = run_full(inputs, trace=False)
    return out
